# revision 2
# baseline (speedup 1.0000x reference)
"""BiLinearAttention TRN2 kernel: (out, score) = attention(query, key, value, W, mask).

  score = softmax((query @ W) @ key^T + mask)   [softmax over s]
  out   = score @ value

Sharding: 8 NeuronCores, core = (batch b = core//2, query-half h = core%2).
Each core computes a [1024, 2048] score block + [1024, 1024] output block.

Per-core dataflow (all on-chip, fp32 score path, bf16 hi/lo split AV path):
  phase 1: QWt[dk, l] = W^T-free matmul (lhsT=W tiles, rhs=Q^T) fp32
  per l-tile: S chunks in PSUM (fp32) -> rowmax -> exp(bias=-max, rowsum via
    accum_out) -> P = E/rowsum -> DMA out -> PE-transpose -> bf16 hi/lo ->
    O = Phi@Vhi + Plo@Vhi + Phi@Vlo (fp32 PSUM accum).
"""
from contextlib import ExitStack

import numpy as np

import concourse.bass as bass
import concourse.mybir as mybir
import concourse.tile as tile
from concourse import bacc
from concourse.bass_utils import run_bass_kernel_spmd
from concourse.masks import make_identity

F32 = mybir.dt.float32
BF16 = mybir.dt.bfloat16
AF = mybir.ActivationFunctionType
AX = mybir.AxisListType
OP = mybir.AluOpType

N_CORES = 8


def _build_program(L, S, DQ, DK, DV, apply_mask, num_devices=N_CORES):
    CH = 512                      # psum chunk width (fp32 bank)
    LT = L // 128                 # l-tiles
    NCH = S // CH                 # s chunks per score row
    KQ = DQ // 128                # contraction tiles for QW
    KD = DK // 128                # dk tiles (partition tiles of QWt)
    SJ = S // 128                 # s-tiles (contraction tiles for AV)
    LCH = max(1, L // CH)         # l chunks in phase 1
    LCW = min(L, CH)
    OCH = max(1, DV // CH)
    OCW = min(DV, CH)

    nc = bacc.Bacc("TRN2", target_bir_lowering=False, debug=False,
                   num_devices=num_devices)

    qT = nc.dram_tensor("qT", [DQ, L], F32, kind="ExternalInput").ap()
    w = nc.dram_tensor("w", [DQ, DK], F32, kind="ExternalInput").ap()
    kT = nc.dram_tensor("kT", [DK, S], F32, kind="ExternalInput").ap()
    v = nc.dram_tensor("v", [S, DV], F32, kind="ExternalInput").ap()
    if apply_mask:
        maskf = nc.dram_tensor("maskf", [1, S], F32, kind="ExternalInput").ap()
    p = nc.dram_tensor("p", [L, S], F32, kind="ExternalOutput").ap()
    o = nc.dram_tensor("o", [L, DV], F32, kind="ExternalOutput").ap()

    with ExitStack() as ctx:
        tc = ctx.enter_context(tile.TileContext(nc))

        const = ctx.enter_context(tc.tile_pool(name="const", bufs=1))
        vh = ctx.enter_context(tc.tile_pool(name="vh", bufs=1))
        ktp = ctx.enter_context(tc.tile_pool(name="ktp", bufs=1))
        qwtp = ctx.enter_context(tc.tile_pool(name="qwtp", bufs=1))
        stag = ctx.enter_context(tc.tile_pool(name="stag", bufs=2))

        ident = const.tile([128, 128], F32, tag="ident")
        make_identity(nc, ident[:])
        if apply_mask:
            # broadcast mask row to all 128 partitions via K=1 matmul with ones
            mrow = const.tile([1, S], F32, tag="mrow")
            nc.sync.dma_start(mrow[:], maskf[:, :])
            ones = const.tile([1, 128], F32, tag="ones")
            nc.vector.memset(ones[:], 1.0)
            m128 = const.tile([128, S], F32, tag="m128")
            with tc.tile_pool(name="ps_m", bufs=2, space="PSUM") as ps_m:
                for c in range(S // CH):
                    pm = ps_m.tile([128, CH], F32, name=f"pm{c}", tag="pm")
                    nc.tensor.matmul(pm[:], ones[:, :], mrow[:, c * CH:(c + 1) * CH],
                                     start=True, stop=True)
                    nc.vector.tensor_copy(m128[:, c * CH:(c + 1) * CH], pm[:])

        # ---- kT resident tiles (used throughout phase 2) ----
        kt_tiles = []
        for k in range(KD):
            t = ktp.tile([128, S], F32, tag=f"kt{k}", name=f"kt{k}")
            nc.sync.dma_start(t[:], kT[k * 128:(k + 1) * 128, :])
            kt_tiles.append(t)

        # ---- V load + bf16 hi/lo split (overlaps phase 1) ----
        vhi, vlo = [], []
        for j in range(SJ):
            st = stag.tile([128, DV], F32, tag="stag", name=f"vstag{j}")
            nc.sync.dma_start(st[:], v[j * 128:(j + 1) * 128, :])
            hi = vh.tile([128, DV], BF16, tag=f"vhi{j}", name=f"vhi{j}")
            nc.vector.tensor_copy(hi[:], st[:])
            lo = vh.tile([128, DV], BF16, tag=f"vlo{j}", name=f"vlo{j}")
            nc.vector.tensor_tensor(lo[:], st[:], hi[:], op=OP.subtract)
            vhi.append(hi)
            vlo.append(lo)

        # ---- phase 1: QWt[dk, l] ----
        qwt = [qwtp.tile([128, L], F32, tag=f"qwt{d}", name=f"qwt{d}")
               for d in range(KD)]

        with (
            tc.tile_pool(name="wp", bufs=1) as wp,
            tc.tile_pool(name="ps1", bufs=8, space="PSUM") as ps1,
        ):
            w_tiles = []
            for k in range(KQ):
                t = wp.tile([128, DK], F32, tag=f"w{k}", name=f"w{k}")
                nc.sync.dma_start(t[:], w[k * 128:(k + 1) * 128, :])
                w_tiles.append(t)

            grp = max(1, 8 // LCH)
            for g0 in range(0, KD, grp):
                dks = range(g0, min(g0 + grp, KD))
                chunks = {}
                for d in dks:
                    for c in range(LCH):
                        chunks[(d, c)] = ps1.tile([128, LCW], F32,
                                                  name=f"ps1_{d}_{c}", tag="ps1")
                for k in range(KQ):
                    qtk = stag.tile([128, L], F32, tag="stag", name=f"qtk{g0}_{k}")
                    nc.sync.dma_start(qtk[:], qT[k * 128:(k + 1) * 128, :])
                    for d in dks:
                        for c in range(LCH):
                            nc.tensor.matmul(
                                chunks[(d, c)][:],
                                w_tiles[k][:, d * 128:(d + 1) * 128],
                                qtk[:, c * LCW:(c + 1) * LCW],
                                start=(k == 0), stop=(k == KQ - 1),
                            )
                for d in dks:
                    for c in range(LCH):
                        nc.vector.tensor_copy(
                            qwt[d][:, c * LCW:(c + 1) * LCW], chunks[(d, c)][:])

        # ---- phase 2/3 fused, per l-tile ----
        with (
            tc.tile_pool(name="ep", bufs=2) as ep,
            tc.tile_pool(name="ptp", bufs=2) as ptp,
            tc.tile_pool(name="op", bufs=2) as op_pool,
            tc.tile_pool(name="scp", bufs=2) as scp,
            tc.tile_pool(name="ps_s", bufs=5, space="PSUM") as ps_s,
            tc.tile_pool(name="ps_t", bufs=2, space="PSUM") as ps_t,
            tc.tile_pool(name="ps_o", bufs=1, space="PSUM") as ps_o,
        ):
            for i in range(LT):
                lsl = slice(i * 128, (i + 1) * 128)
                schunks = [ps_s.tile([128, CH], F32, name=f"ssc{i}_{c}", tag="ssc")
                           for c in range(NCH)]
                for k in range(KD):
                    for c in range(NCH):
                        nc.tensor.matmul(
                            schunks[c][:],
                            qwt[k][:, lsl],
                            kt_tiles[k][:, c * CH:(c + 1) * CH],
                            start=(k == 0), stop=(k == KD - 1),
                        )
                scal = scp.tile([128, 8], F32, tag="scal", name=f"scal{i}")
                for c in range(NCH):
                    nc.vector.reduce_max(scal[:, c:c + 1], schunks[c][:],
                                         axis=AX.X, negate=True)
                nm = scal[:, NCH:NCH + 1]
                if NCH == 1:
                    nc.vector.tensor_copy(nm, scal[:, 0:1])
                else:
                    nc.vector.tensor_tensor(nm, scal[:, 0:1], scal[:, 1:2], op=OP.min)
                    for c in range(2, NCH):
                        nc.vector.tensor_tensor(nm, nm, scal[:, c:c + 1], op=OP.min)

                e = ep.tile([128, S], F32, tag="e", name=f"e{i}")
                sums = scp.tile([128, NCH + 2], F32, tag="sums", name=f"sums{i}")
                for c in range(NCH):
                    nc.scalar.activation(e[:, c * CH:(c + 1) * CH], schunks[c][:],
                                         AF.Exp, bias=nm,
                                         accum_out=sums[:, c:c + 1])
                if apply_mask:
                    # multiplicative mask after exp == additive -inf mask
                    # pre-softmax (invalid only for fully-masked rows)
                    nc.vector.tensor_tensor(e[:], e[:], m128[:], op=OP.mult)
                    nc.vector.reduce_sum(sums[:, NCH:NCH + 1], e[:], axis=AX.X)
                    tot = sums[:, NCH:NCH + 1]
                else:
                    tot = sums[:, NCH:NCH + 1]
                    if NCH == 1:
                        tot = sums[:, 0:1]
                    else:
                        nc.vector.tensor_tensor(tot, sums[:, 0:1], sums[:, 1:2],
                                                op=OP.add)
                        for c in range(2, NCH):
                            nc.vector.tensor_tensor(tot, tot, sums[:, c:c + 1],
                                                    op=OP.add)
                rinv = sums[:, NCH + 1:NCH + 2]
                nc.vector.reciprocal(rinv, tot)
                nc.vector.tensor_scalar_mul(e[:], e[:], rinv)
                nc.sync.dma_start(p[lsl, :], e[:])

                pthi, ptlo = [], []
                for j in range(SJ):
                    pst = ps_t.tile([128, 128], F32, name=f"pst{i}_{j}", tag="pst")
                    nc.tensor.transpose(pst[:], e[:, j * 128:(j + 1) * 128], ident[:])
                    hi = ptp.tile([128, 128], BF16, tag=f"pthi{j}", name=f"pthi{i}_{j}")
                    nc.vector.tensor_copy(hi[:], pst[:])
                    lo = ptp.tile([128, 128], BF16, tag=f"ptlo{j}", name=f"ptlo{i}_{j}")
                    nc.vector.tensor_tensor(lo[:], pst[:], hi[:], op=OP.subtract)
                    pthi.append(hi)
                    ptlo.append(lo)

                for c in range(OCH):
                    osl = slice(c * OCW, (c + 1) * OCW)
                    po = ps_o.tile([128, OCW], F32, name=f"po{i}_{c}", tag="po")
                    for j in range(SJ):
                        nc.tensor.matmul(po[:], pthi[j][:], vhi[j][:, osl],
                                         start=(j == 0), stop=False)
                        nc.tensor.matmul(po[:], ptlo[j][:], vhi[j][:, osl],
                                         start=False, stop=False)
                        nc.tensor.matmul(po[:], pthi[j][:], vlo[j][:, osl],
                                         start=False, stop=(j == SJ - 1))
                    ot = op_pool.tile([128, OCW], F32, tag="ot", name=f"ot{i}_{c}")
                    nc.vector.tensor_copy(ot[:], po[:])
                    nc.sync.dma_start(o[lsl, osl], ot[:])

    nc.compile()
    return nc


_PROGRAM_CACHE = {}


def _get_program(L, S, DQ, DK, DV, apply_mask):
    key = (L, S, DQ, DK, DV, apply_mask)
    if key not in _PROGRAM_CACHE:
        _PROGRAM_CACHE[key] = _build_program(L, S, DQ, DK, DV, apply_mask)
    return _PROGRAM_CACHE[key]


def _run(query, key, value, W, mask, trace=False):
    n, l, dq = query.shape
    _, s, dk = key.shape
    dv = value.shape[2]
    assert n * 2 == N_CORES and l % (2 * 128) == 0
    L = l // 2

    apply_mask = not bool(np.all(mask))
    nc = _get_program(L, s, dq, dk, dv, apply_mask)

    in_maps = []
    for core in range(N_CORES):
        b, h = divmod(core, 2)
        im = {
            "qT": np.ascontiguousarray(
                query[b, h * L:(h + 1) * L, :].T).astype(np.float32, copy=False),
            "w": np.ascontiguousarray(W).astype(np.float32, copy=False),
            "kT": np.ascontiguousarray(key[b].T).astype(np.float32, copy=False),
            "v": np.ascontiguousarray(value[b]).astype(np.float32, copy=False),
        }
        if apply_mask:
            im["maskf"] = np.ascontiguousarray(
                mask[b].astype(np.float32)[None, :])
        in_maps.append(im)

    res = run_bass_kernel_spmd(nc, in_maps, core_ids=list(range(N_CORES)),
                               trace=trace)

    score = np.empty((n, l, s), dtype=np.float32)
    out = np.empty((n, l, dv), dtype=np.float32)
    for core in range(N_CORES):
        b, h = divmod(core, 2)
        score[b, h * L:(h + 1) * L, :] = res.results[core]["p"]
        out[b, h * L:(h + 1) * L, :] = res.results[core]["o"]
    return (out, score), res


def kernel(query, key, value, W, mask):
    (out, score), _ = _run(np.asarray(query), np.asarray(key), np.asarray(value),
                           np.asarray(W), np.asarray(mask))
    return (out, score)


# revision 5
# speedup vs baseline: 1.0846x; 1.0846x over previous
"""BiLinearAttention TRN2 kernel: (out, score) = attention(query, key, value, W, mask).

  score = softmax((query @ W) @ key^T + mask)   [softmax over s]
  out   = score @ value

Sharding: 8 NeuronCores, core = (batch b = core//2, query-half h = core%2).
Each core computes a [1024, 2048] score block + [1024, 1024] output block.

Per-core dataflow (all on-chip, fp32 score path, bf16 hi/lo split AV path):
  phase 1: QWt[dk, l] = W^T-free matmul (lhsT=W tiles, rhs=Q^T) fp32
  per l-tile: S chunks in PSUM (fp32) -> rowmax -> exp(bias=-max, rowsum via
    accum_out) -> P = E/rowsum -> DMA out -> PE-transpose -> bf16 hi/lo ->
    O = Phi@Vhi + Plo@Vhi + Phi@Vlo (fp32 PSUM accum).
"""
from contextlib import ExitStack

import numpy as np

import concourse.bass as bass
import concourse.mybir as mybir
import concourse.tile as tile
from concourse import bacc
from concourse.bass_utils import run_bass_kernel_spmd
from concourse.masks import make_identity

F32 = mybir.dt.float32
BF16 = mybir.dt.bfloat16
AF = mybir.ActivationFunctionType
AX = mybir.AxisListType
OP = mybir.AluOpType

N_CORES = 8


def _build_program(L, S, DQ, DK, DV, apply_mask, num_devices=N_CORES):
    CH = 512                      # psum chunk width (fp32 bank)
    LT = L // 128                 # l-tiles
    NCH = S // CH                 # s chunks per score row
    KQ = DQ // 128                # contraction tiles for QW
    KD = DK // 128                # dk tiles (partition tiles of QWt)
    SJ = S // 128                 # s-tiles (contraction tiles for AV)
    LCH = max(1, L // CH)         # l chunks in phase 1
    LCW = min(L, CH)
    OCH = max(1, DV // CH)
    OCW = min(DV, CH)

    nc = bacc.Bacc("TRN2", target_bir_lowering=False, debug=False,
                   num_devices=num_devices)

    qT = nc.dram_tensor("qT", [DQ, L], F32, kind="ExternalInput").ap()
    w = nc.dram_tensor("w", [DQ, DK], F32, kind="ExternalInput").ap()
    kT = nc.dram_tensor("kT", [DK, S], F32, kind="ExternalInput").ap()
    v = nc.dram_tensor("v", [S, DV], F32, kind="ExternalInput").ap()
    if apply_mask:
        maskf = nc.dram_tensor("maskf", [1, S], F32, kind="ExternalInput").ap()
    p = nc.dram_tensor("p", [L, S], F32, kind="ExternalOutput").ap()
    o = nc.dram_tensor("o", [L, DV], F32, kind="ExternalOutput").ap()

    with ExitStack() as ctx:
        tc = ctx.enter_context(tile.TileContext(nc))

        const = ctx.enter_context(tc.tile_pool(name="const", bufs=1))
        vh = ctx.enter_context(tc.tile_pool(name="vh", bufs=1))
        ktp = ctx.enter_context(tc.tile_pool(name="ktp", bufs=1))
        qwtp = ctx.enter_context(tc.tile_pool(name="qwtp", bufs=1))
        stag = ctx.enter_context(tc.tile_pool(name="stag", bufs=2))
        vstag = ctx.enter_context(tc.tile_pool(name="vstag", bufs=2))

        ident = const.tile([128, 128], F32, tag="ident")
        make_identity(nc, ident[:])
        if apply_mask:
            # broadcast mask row to all 128 partitions via K=1 matmul with ones
            mrow = const.tile([1, S], F32, tag="mrow")
            nc.sync.dma_start(mrow[:], maskf[:, :])
            ones = const.tile([1, 128], F32, tag="ones")
            nc.vector.memset(ones[:], 1.0)
            m128 = const.tile([128, S], F32, tag="m128")
            with tc.tile_pool(name="ps_m", bufs=2, space="PSUM") as ps_m:
                for c in range(S // CH):
                    pm = ps_m.tile([128, CH], F32, name=f"pm{c}", tag="pm")
                    nc.tensor.matmul(pm[:], ones[:, :], mrow[:, c * CH:(c + 1) * CH],
                                     start=True, stop=True)
                    nc.vector.tensor_copy(m128[:, c * CH:(c + 1) * CH], pm[:])

        # ---- phase 1: QWt[dk, l] ----
        qwt = [qwtp.tile([128, L], F32, tag=f"qwt{d}", name=f"qwt{d}")
               for d in range(KD)]

        with (
            tc.tile_pool(name="wp", bufs=1) as wp,
            tc.tile_pool(name="ps1", bufs=8, space="PSUM") as ps1,
        ):
            w_tiles = []
            for k in range(KQ):
                t = wp.tile([128, DK], F32, tag=f"w{k}", name=f"w{k}")
                nc.sync.dma_start(t[:], w[k * 128:(k + 1) * 128, :])
                w_tiles.append(t)

            kt_tiles = []
            vhi, vlo = [], []

            grp = max(1, 8 // LCH)
            groups = list(range(0, KD, grp))
            for g0 in groups:
                dks = range(g0, min(g0 + grp, KD))
                chunks = {}
                for d in dks:
                    for c in range(LCH):
                        chunks[(d, c)] = ps1.tile([128, LCW], F32,
                                                  name=f"ps1_{d}_{c}", tag="ps1")
                for k in range(KQ):
                    qtk = stag.tile([128, L], F32, tag="stag", name=f"qtk{g0}_{k}")
                    nc.gpsimd.dma_start(qtk[:], qT[k * 128:(k + 1) * 128, :])
                    for d in dks:
                        for c in range(LCH):
                            nc.tensor.matmul(
                                chunks[(d, c)][:],
                                w_tiles[k][:, d * 128:(d + 1) * 128],
                                qtk[:, c * LCW:(c + 1) * LCW],
                                start=(k == 0), stop=(k == KQ - 1),
                            )
                if g0 == groups[0]:
                    # bulk loads for later phases queue behind group 0's inputs
                    for k in range(KD):
                        t = ktp.tile([128, S], F32, tag=f"kt{k}", name=f"kt{k}")
                        nc.sync.dma_start(t[:], kT[k * 128:(k + 1) * 128, :])
                        kt_tiles.append(t)
                    VST = min(DV, 512)
                    for j in range(SJ):
                        hi = vh.tile([128, DV], BF16, tag=f"vhi{j}", name=f"vhi{j}")
                        lo = vh.tile([128, DV], BF16, tag=f"vlo{j}", name=f"vlo{j}")
                        for vc in range(DV // VST):
                            vsl = slice(vc * VST, (vc + 1) * VST)
                            st = vstag.tile([128, VST], F32, tag="vstag",
                                            name=f"vstag{j}_{vc}")
                            nc.sync.dma_start(st[:], v[j * 128:(j + 1) * 128, vsl])
                            nc.vector.tensor_copy(hi[:, vsl], st[:])
                            nc.vector.tensor_tensor(lo[:, vsl], st[:], hi[:, vsl],
                                                    op=OP.subtract)
                        vhi.append(hi)
                        vlo.append(lo)
                for d in dks:
                    for c in range(LCH):
                        nc.vector.tensor_copy(
                            qwt[d][:, c * LCW:(c + 1) * LCW], chunks[(d, c)][:])

        # ---- phase 2/3 fused, per l-tile ----
        with (
            tc.tile_pool(name="ep", bufs=2) as ep,
            tc.tile_pool(name="ptp", bufs=1) as ptp,
            tc.tile_pool(name="op", bufs=2) as op_pool,
            tc.tile_pool(name="scp", bufs=2) as scp,
            tc.tile_pool(name="ps_s", bufs=5, space="PSUM") as ps_s,
            tc.tile_pool(name="ps_t", bufs=2, space="PSUM") as ps_t,
            tc.tile_pool(name="ps_o", bufs=1, space="PSUM") as ps_o,
        ):
            for i in range(LT):
                lsl = slice(i * 128, (i + 1) * 128)
                schunks = [ps_s.tile([128, CH], F32, name=f"ssc{i}_{c}", tag="ssc")
                           for c in range(NCH)]
                for k in range(KD):
                    for c in range(NCH):
                        nc.tensor.matmul(
                            schunks[c][:],
                            qwt[k][:, lsl],
                            kt_tiles[k][:, c * CH:(c + 1) * CH],
                            start=(k == 0), stop=(k == KD - 1),
                        )
                scal = scp.tile([128, 8], F32, tag="scal", name=f"scal{i}")
                for c in range(NCH):
                    nc.vector.reduce_max(scal[:, c:c + 1], schunks[c][:],
                                         axis=AX.X, negate=True)
                nm = scal[:, NCH:NCH + 1]
                if NCH == 1:
                    nc.vector.tensor_copy(nm, scal[:, 0:1])
                else:
                    nc.vector.tensor_tensor(nm, scal[:, 0:1], scal[:, 1:2], op=OP.min)
                    for c in range(2, NCH):
                        nc.vector.tensor_tensor(nm, nm, scal[:, c:c + 1], op=OP.min)

                e = ep.tile([128, S], F32, tag="e", name=f"e{i}")
                sums = scp.tile([128, NCH + 2], F32, tag="sums", name=f"sums{i}")
                for c in range(NCH):
                    nc.scalar.activation(e[:, c * CH:(c + 1) * CH], schunks[c][:],
                                         AF.Exp, bias=nm,
                                         accum_out=sums[:, c:c + 1])
                if apply_mask:
                    # multiplicative mask after exp == additive -inf mask
                    # pre-softmax (invalid only for fully-masked rows)
                    nc.vector.tensor_tensor(e[:], e[:], m128[:], op=OP.mult)
                    nc.vector.reduce_sum(sums[:, NCH:NCH + 1], e[:], axis=AX.X)
                    tot = sums[:, NCH:NCH + 1]
                else:
                    tot = sums[:, NCH:NCH + 1]
                    if NCH == 1:
                        tot = sums[:, 0:1]
                    else:
                        nc.vector.tensor_tensor(tot, sums[:, 0:1], sums[:, 1:2],
                                                op=OP.add)
                        for c in range(2, NCH):
                            nc.vector.tensor_tensor(tot, tot, sums[:, c:c + 1],
                                                    op=OP.add)
                rinv = sums[:, NCH + 1:NCH + 2]
                nc.vector.reciprocal(rinv, tot)
                nc.vector.tensor_scalar_mul(e[:], e[:], rinv)
                nc.sync.dma_start(p[lsl, :], e[:])

                pthi, ptlo = [], []
                for j in range(SJ):
                    pst = ps_t.tile([128, 128], F32, name=f"pst{i}_{j}", tag="pst")
                    nc.tensor.transpose(pst[:], e[:, j * 128:(j + 1) * 128], ident[:])
                    hi = ptp.tile([128, 128], BF16, tag=f"pthi{j}", name=f"pthi{i}_{j}")
                    nc.vector.tensor_copy(hi[:], pst[:])
                    lo = ptp.tile([128, 128], BF16, tag=f"ptlo{j}", name=f"ptlo{i}_{j}")
                    nc.vector.tensor_tensor(lo[:], pst[:], hi[:], op=OP.subtract)
                    pthi.append(hi)
                    ptlo.append(lo)

                for c in range(OCH):
                    osl = slice(c * OCW, (c + 1) * OCW)
                    po = ps_o.tile([128, OCW], F32, name=f"po{i}_{c}", tag="po")
                    for j in range(SJ):
                        nc.tensor.matmul(po[:], pthi[j][:], vhi[j][:, osl],
                                         start=(j == 0), stop=False)
                        nc.tensor.matmul(po[:], ptlo[j][:], vhi[j][:, osl],
                                         start=False, stop=False)
                        nc.tensor.matmul(po[:], pthi[j][:], vlo[j][:, osl],
                                         start=False, stop=(j == SJ - 1))
                    ot = op_pool.tile([128, OCW], F32, tag="ot", name=f"ot{i}_{c}")
                    nc.vector.tensor_copy(ot[:], po[:])
                    nc.sync.dma_start(o[lsl, osl], ot[:])

    nc.compile()
    return nc


_PROGRAM_CACHE = {}


def _get_program(L, S, DQ, DK, DV, apply_mask):
    key = (L, S, DQ, DK, DV, apply_mask)
    if key not in _PROGRAM_CACHE:
        _PROGRAM_CACHE[key] = _build_program(L, S, DQ, DK, DV, apply_mask)
    return _PROGRAM_CACHE[key]


def _run(query, key, value, W, mask, trace=False):
    n, l, dq = query.shape
    _, s, dk = key.shape
    dv = value.shape[2]
    assert n * 2 == N_CORES and l % (2 * 128) == 0
    L = l // 2

    apply_mask = not bool(np.all(mask))
    nc = _get_program(L, s, dq, dk, dv, apply_mask)

    in_maps = []
    for core in range(N_CORES):
        b, h = divmod(core, 2)
        im = {
            "qT": np.ascontiguousarray(
                query[b, h * L:(h + 1) * L, :].T).astype(np.float32, copy=False),
            "w": np.ascontiguousarray(W).astype(np.float32, copy=False),
            "kT": np.ascontiguousarray(key[b].T).astype(np.float32, copy=False),
            "v": np.ascontiguousarray(value[b]).astype(np.float32, copy=False),
        }
        if apply_mask:
            im["maskf"] = np.ascontiguousarray(
                mask[b].astype(np.float32)[None, :])
        in_maps.append(im)

    res = run_bass_kernel_spmd(nc, in_maps, core_ids=list(range(N_CORES)),
                               trace=trace)

    score = np.empty((n, l, s), dtype=np.float32)
    out = np.empty((n, l, dv), dtype=np.float32)
    for core in range(N_CORES):
        b, h = divmod(core, 2)
        score[b, h * L:(h + 1) * L, :] = res.results[core]["p"]
        out[b, h * L:(h + 1) * L, :] = res.results[core]["o"]
    return (out, score), res


def kernel(query, key, value, W, mask):
    (out, score), _ = _run(np.asarray(query), np.asarray(key), np.asarray(value),
                           np.asarray(W), np.asarray(mask))
    return (out, score)


# revision 7
# speedup vs baseline: 1.2823x; 1.1824x over previous
"""BiLinearAttention TRN2 kernel: (out, score) = attention(query, key, value, W, mask).

  score = softmax((query @ W) @ key^T + mask)   [softmax over s]
  out   = score @ value

Sharding: 8 NeuronCores, core = (batch b = core//2, query-half h = core%2).
Each core computes a [1024, 2048] score block + [1024, 1024] output block.

Per-core dataflow (all on-chip, fp32 score path, bf16 hi/lo split AV path):
  phase 1: QWt[dk, l] = W^T-free matmul (lhsT=W tiles, rhs=Q^T) fp32
  per l-tile: S chunks in PSUM (fp32) -> rowmax -> exp(bias=-max, rowsum via
    accum_out) -> P = E/rowsum -> DMA out -> PE-transpose -> bf16 hi/lo ->
    O = Phi@Vhi + Plo@Vhi + Phi@Vlo (fp32 PSUM accum).
"""
from contextlib import ExitStack

import numpy as np

import concourse.bass as bass
import concourse.mybir as mybir
import concourse.tile as tile
from concourse import bacc
from concourse.bass_utils import run_bass_kernel_spmd
from concourse.masks import make_identity

F32 = mybir.dt.float32
F32R = mybir.dt.float32r
BF16 = mybir.dt.bfloat16
AF = mybir.ActivationFunctionType
AX = mybir.AxisListType
OP = mybir.AluOpType

N_CORES = 8


def _build_program(L, S, DQ, DK, DV, apply_mask, num_devices=N_CORES):
    CH = 512                      # psum chunk width (fp32 bank)
    LT = L // 128                 # l-tiles
    NCH = S // CH                 # s chunks per score row
    KQ = DQ // 128                # contraction tiles for QW
    KD = DK // 128                # dk tiles (partition tiles of QWt)
    SJ = S // 128                 # s-tiles (contraction tiles for AV)
    LCH = max(1, L // CH)         # l chunks in phase 1
    LCW = min(L, CH)
    OCH = max(1, DV // CH)
    OCW = min(DV, CH)

    nc = bacc.Bacc("TRN2", target_bir_lowering=False, debug=False,
                   num_devices=num_devices)

    qT = nc.dram_tensor("qT", [DQ, L], F32, kind="ExternalInput").ap()
    w = nc.dram_tensor("w", [DQ, DK], F32, kind="ExternalInput").ap()
    kT = nc.dram_tensor("kT", [DK, S], F32, kind="ExternalInput").ap()
    v = nc.dram_tensor("v", [S, DV], F32, kind="ExternalInput").ap()
    if apply_mask:
        maskf = nc.dram_tensor("maskf", [1, S], F32, kind="ExternalInput").ap()
    p = nc.dram_tensor("p", [L, S], F32, kind="ExternalOutput").ap()
    o = nc.dram_tensor("o", [L, DV], F32, kind="ExternalOutput").ap()

    with ExitStack() as ctx:
        tc = ctx.enter_context(tile.TileContext(nc))

        const = ctx.enter_context(tc.tile_pool(name="const", bufs=1))
        vh = ctx.enter_context(tc.tile_pool(name="vh", bufs=1))
        ktp = ctx.enter_context(tc.tile_pool(name="ktp", bufs=1))
        qwtp = ctx.enter_context(tc.tile_pool(name="qwtp", bufs=1))
        stag = ctx.enter_context(tc.tile_pool(name="stag", bufs=2))
        vstag = ctx.enter_context(tc.tile_pool(name="vstag", bufs=2))

        ident = const.tile([128, 128], F32, tag="ident")
        make_identity(nc, ident[:])
        if apply_mask:
            # broadcast mask row to all 128 partitions via K=1 matmul with ones
            mrow = const.tile([1, S], F32, tag="mrow")
            nc.sync.dma_start(mrow[:], maskf[:, :])
            ones = const.tile([1, 128], F32, tag="ones")
            nc.vector.memset(ones[:], 1.0)
            m128 = const.tile([128, S], F32, tag="m128")
            with tc.tile_pool(name="ps_m", bufs=2, space="PSUM") as ps_m:
                for c in range(S // CH):
                    pm = ps_m.tile([128, CH], F32, name=f"pm{c}", tag="pm")
                    nc.tensor.matmul(pm[:], ones[:, :], mrow[:, c * CH:(c + 1) * CH],
                                     start=True, stop=True)
                    nc.vector.tensor_copy(m128[:, c * CH:(c + 1) * CH], pm[:])

        # ---- phase 1: QWt[dk, l] ----
        qwt = [qwtp.tile([128, L], F32, tag=f"qwt{d}", name=f"qwt{d}")
               for d in range(KD)]

        with (
            tc.tile_pool(name="wp", bufs=1) as wp,
            tc.tile_pool(name="ps1", bufs=8, space="PSUM") as ps1,
        ):
            w_tiles = []
            for k in range(KQ):
                t = wp.tile([128, DK], F32, tag=f"w{k}", name=f"w{k}")
                nc.sync.dma_start(t[:], w[k * 128:(k + 1) * 128, :])
                w_tiles.append(t)

            kt_tiles = []
            vrt = []

            grp = max(1, 8 // LCH)
            groups = list(range(0, KD, grp))
            for g0 in groups:
                dks = range(g0, min(g0 + grp, KD))
                chunks = {}
                for d in dks:
                    for c in range(LCH):
                        chunks[(d, c)] = ps1.tile([128, LCW], F32,
                                                  name=f"ps1_{d}_{c}", tag="ps1")
                for k in range(KQ):
                    qtk = stag.tile([128, L], F32, tag="stag", name=f"qtk{g0}_{k}")
                    nc.gpsimd.dma_start(qtk[:], qT[k * 128:(k + 1) * 128, :])
                    for d in dks:
                        for c in range(LCH):
                            nc.tensor.matmul(
                                chunks[(d, c)][:],
                                w_tiles[k][:, d * 128:(d + 1) * 128],
                                qtk[:, c * LCW:(c + 1) * LCW],
                                start=(k == 0), stop=(k == KQ - 1),
                            )
                if g0 == groups[0]:
                    # bulk loads for later phases queue behind group 0's inputs
                    for k in range(KD):
                        t = ktp.tile([128, S], F32, tag=f"kt{k}", name=f"kt{k}")
                        nc.sync.dma_start(t[:], kT[k * 128:(k + 1) * 128, :])
                        kt_tiles.append(t)
                    VST = min(DV, 512)
                    for j in range(SJ):
                        vr = vh.tile([128, DV], F32R, tag=f"vr{j}", name=f"vr{j}")
                        for vc in range(DV // VST):
                            vsl = slice(vc * VST, (vc + 1) * VST)
                            st = vstag.tile([128, VST], F32, tag="vstag",
                                            name=f"vstag{j}_{vc}")
                            nc.sync.dma_start(st[:], v[j * 128:(j + 1) * 128, vsl])
                            nc.vector.tensor_copy(vr[:, vsl], st[:])
                        vrt.append(vr)
                for d in dks:
                    for c in range(LCH):
                        nc.vector.tensor_copy(
                            qwt[d][:, c * LCW:(c + 1) * LCW], chunks[(d, c)][:])

        # ---- phase 2/3 fused, per l-tile ----
        with (
            tc.tile_pool(name="ep", bufs=2) as ep,
            tc.tile_pool(name="ptp", bufs=1) as ptp,
            tc.tile_pool(name="op", bufs=2) as op_pool,
            tc.tile_pool(name="scp", bufs=2) as scp,
            tc.tile_pool(name="ps_s", bufs=4, space="PSUM") as ps_s,
            tc.tile_pool(name="ps_t", bufs=2, space="PSUM") as ps_t,
            tc.tile_pool(name="ps_o", bufs=2, space="PSUM") as ps_o,
        ):
            for i in range(LT):
                lsl = slice(i * 128, (i + 1) * 128)
                schunks = [ps_s.tile([128, CH], F32, name=f"ssc{i}_{c}", tag="ssc")
                           for c in range(NCH)]
                for k in range(KD):
                    for c in range(NCH):
                        nc.tensor.matmul(
                            schunks[c][:],
                            qwt[k][:, lsl],
                            kt_tiles[k][:, c * CH:(c + 1) * CH],
                            start=(k == 0), stop=(k == KD - 1),
                        )
                scal = scp.tile([128, 8], F32, tag="scal", name=f"scal{i}")
                for c in range(NCH):
                    nc.vector.reduce_max(scal[:, c:c + 1], schunks[c][:],
                                         axis=AX.X, negate=True)
                nm = scal[:, NCH:NCH + 1]
                if NCH == 1:
                    nc.vector.tensor_copy(nm, scal[:, 0:1])
                else:
                    nc.vector.tensor_tensor(nm, scal[:, 0:1], scal[:, 1:2], op=OP.min)
                    for c in range(2, NCH):
                        nc.vector.tensor_tensor(nm, nm, scal[:, c:c + 1], op=OP.min)

                e = ep.tile([128, S], F32, tag="e", name=f"e{i}")
                sums = scp.tile([128, NCH + 2], F32, tag="sums", name=f"sums{i}")
                for c in range(NCH):
                    nc.scalar.activation(e[:, c * CH:(c + 1) * CH], schunks[c][:],
                                         AF.Exp, bias=nm,
                                         accum_out=sums[:, c:c + 1])
                if apply_mask:
                    # multiplicative mask after exp == additive -inf mask
                    # pre-softmax (invalid only for fully-masked rows)
                    nc.vector.tensor_tensor(e[:], e[:], m128[:], op=OP.mult)
                    nc.vector.reduce_sum(sums[:, NCH:NCH + 1], e[:], axis=AX.X)
                    tot = sums[:, NCH:NCH + 1]
                else:
                    tot = sums[:, NCH:NCH + 1]
                    if NCH == 1:
                        tot = sums[:, 0:1]
                    else:
                        nc.vector.tensor_tensor(tot, sums[:, 0:1], sums[:, 1:2],
                                                op=OP.add)
                        for c in range(2, NCH):
                            nc.vector.tensor_tensor(tot, tot, sums[:, c:c + 1],
                                                    op=OP.add)
                rinv = sums[:, NCH + 1:NCH + 2]
                nc.vector.reciprocal(rinv, tot)
                nc.vector.tensor_scalar_mul(e[:], e[:], rinv)
                nc.sync.dma_start(p[lsl, :], e[:])

                ptr = []
                for j in range(SJ):
                    pst = ps_t.tile([128, 128], F32, name=f"pst{i}_{j}", tag="pst")
                    nc.tensor.transpose(pst[:], e[:, j * 128:(j + 1) * 128], ident[:])
                    pr = ptp.tile([128, 128], F32R, tag=f"ptr{j}", name=f"ptr{i}_{j}")
                    nc.vector.tensor_copy(pr[:], pst[:])
                    ptr.append(pr)

                pos = [ps_o.tile([128, OCW], F32, name=f"po{i}_{c}", tag="po")
                       for c in range(OCH)]
                for j in range(SJ):
                    for c in range(OCH):
                        nc.tensor.matmul(pos[c][:], ptr[j][:],
                                         vrt[j][:, c * OCW:(c + 1) * OCW],
                                         start=(j == 0), stop=(j == SJ - 1))
                for c in range(OCH):
                    osl = slice(c * OCW, (c + 1) * OCW)
                    ot = op_pool.tile([128, OCW], F32, tag="ot", name=f"ot{i}_{c}")
                    nc.vector.tensor_copy(ot[:], pos[c][:])
                    nc.sync.dma_start(o[lsl, osl], ot[:])

    nc.compile()
    return nc


_PROGRAM_CACHE = {}


def _get_program(L, S, DQ, DK, DV, apply_mask):
    key = (L, S, DQ, DK, DV, apply_mask)
    if key not in _PROGRAM_CACHE:
        _PROGRAM_CACHE[key] = _build_program(L, S, DQ, DK, DV, apply_mask)
    return _PROGRAM_CACHE[key]


def _run(query, key, value, W, mask, trace=False):
    n, l, dq = query.shape
    _, s, dk = key.shape
    dv = value.shape[2]
    assert n * 2 == N_CORES and l % (2 * 128) == 0
    L = l // 2

    apply_mask = not bool(np.all(mask))
    nc = _get_program(L, s, dq, dk, dv, apply_mask)

    in_maps = []
    for core in range(N_CORES):
        b, h = divmod(core, 2)
        im = {
            "qT": np.ascontiguousarray(
                query[b, h * L:(h + 1) * L, :].T).astype(np.float32, copy=False),
            "w": np.ascontiguousarray(W).astype(np.float32, copy=False),
            "kT": np.ascontiguousarray(key[b].T).astype(np.float32, copy=False),
            "v": np.ascontiguousarray(value[b]).astype(np.float32, copy=False),
        }
        if apply_mask:
            im["maskf"] = np.ascontiguousarray(
                mask[b].astype(np.float32)[None, :])
        in_maps.append(im)

    res = run_bass_kernel_spmd(nc, in_maps, core_ids=list(range(N_CORES)),
                               trace=trace)

    score = np.empty((n, l, s), dtype=np.float32)
    out = np.empty((n, l, dv), dtype=np.float32)
    for core in range(N_CORES):
        b, h = divmod(core, 2)
        score[b, h * L:(h + 1) * L, :] = res.results[core]["p"]
        out[b, h * L:(h + 1) * L, :] = res.results[core]["o"]
    return (out, score), res


def kernel(query, key, value, W, mask):
    (out, score), _ = _run(np.asarray(query), np.asarray(key), np.asarray(value),
                           np.asarray(W), np.asarray(mask))
    return (out, score)


# revision 9
# speedup vs baseline: 1.3320x; 1.0387x over previous
"""BiLinearAttention TRN2 kernel: (out, score) = attention(query, key, value, W, mask).

  score = softmax((query @ W) @ key^T + mask)   [softmax over s]
  out   = score @ value

Sharding: 8 NeuronCores, core = (batch b = core//2, query-half h = core%2).
Each core computes a [1024, 2048] score block + [1024, 1024] output block.

Per-core dataflow (all on-chip, fp32 score path, bf16 hi/lo split AV path):
  phase 1: QWt[dk, l] = W^T-free matmul (lhsT=W tiles, rhs=Q^T) fp32
  per l-tile: S chunks in PSUM (fp32) -> rowmax -> exp(bias=-max, rowsum via
    accum_out) -> P = E/rowsum -> DMA out -> PE-transpose -> bf16 hi/lo ->
    O = Phi@Vhi + Plo@Vhi + Phi@Vlo (fp32 PSUM accum).
"""
from contextlib import ExitStack

import numpy as np

import concourse.bass as bass
import concourse.mybir as mybir
import concourse.tile as tile
from concourse import bacc
from concourse.bass_utils import run_bass_kernel_spmd
from concourse.masks import make_identity

F32 = mybir.dt.float32
F32R = mybir.dt.float32r
BF16 = mybir.dt.bfloat16
AF = mybir.ActivationFunctionType
AX = mybir.AxisListType
OP = mybir.AluOpType

N_CORES = 8


def _build_program(L, S, DQ, DK, DV, apply_mask, num_devices=N_CORES):
    CH = 512                      # psum chunk width (fp32 bank)
    LT = L // 128                 # l-tiles
    NCH = S // CH                 # s chunks per score row
    KQ = DQ // 128                # contraction tiles for QW
    KD = DK // 128                # dk tiles (partition tiles of QWt)
    SJ = S // 128                 # s-tiles (contraction tiles for AV)
    LCH = max(1, L // CH)         # l chunks in phase 1
    LCW = min(L, CH)
    OCH = max(1, DV // CH)
    OCW = min(DV, CH)

    nc = bacc.Bacc("TRN2", target_bir_lowering=False, debug=False,
                   num_devices=num_devices)

    qT = nc.dram_tensor("qT", [DQ, L], F32, kind="ExternalInput").ap()
    w = nc.dram_tensor("w", [DQ, DK], F32, kind="ExternalInput").ap()
    kT = nc.dram_tensor("kT", [DK, S], F32, kind="ExternalInput").ap()
    v = nc.dram_tensor("v", [S, DV], F32, kind="ExternalInput").ap()
    if apply_mask:
        maskf = nc.dram_tensor("maskf", [1, S], F32, kind="ExternalInput").ap()
    p = nc.dram_tensor("p", [L, S], F32, kind="ExternalOutput").ap()
    o = nc.dram_tensor("o", [L, DV], F32, kind="ExternalOutput").ap()

    with ExitStack() as ctx:
        tc = ctx.enter_context(tile.TileContext(nc))

        const = ctx.enter_context(tc.tile_pool(name="const", bufs=1))
        vh = ctx.enter_context(tc.tile_pool(name="vh", bufs=1))
        ktp = ctx.enter_context(tc.tile_pool(name="ktp", bufs=1))
        qwtp = ctx.enter_context(tc.tile_pool(name="qwtp", bufs=1))
        stag = ctx.enter_context(tc.tile_pool(name="stag", bufs=2))
        vstag = ctx.enter_context(tc.tile_pool(name="vstag", bufs=2))

        ident = const.tile([128, 128], F32, tag="ident")
        make_identity(nc, ident[:])
        if apply_mask:
            # broadcast mask row to all 128 partitions via K=1 matmul with ones
            mrow = const.tile([1, S], F32, tag="mrow")
            nc.sync.dma_start(mrow[:], maskf[:, :])
            ones = const.tile([1, 128], F32, tag="ones")
            nc.vector.memset(ones[:], 1.0)
            m128 = const.tile([128, S], F32, tag="m128")
            with tc.tile_pool(name="ps_m", bufs=2, space="PSUM") as ps_m:
                for c in range(S // CH):
                    pm = ps_m.tile([128, CH], F32, name=f"pm{c}", tag="pm")
                    nc.tensor.matmul(pm[:], ones[:, :], mrow[:, c * CH:(c + 1) * CH],
                                     start=True, stop=True)
                    nc.vector.tensor_copy(m128[:, c * CH:(c + 1) * CH], pm[:])

        # ---- phase 1: QWt[dk, l] ----
        qwt = [qwtp.tile([128, L], F32, tag=f"qwt{d}", name=f"qwt{d}")
               for d in range(KD)]

        with (
            tc.tile_pool(name="wp", bufs=1) as wp,
            tc.tile_pool(name="ps1", bufs=8, space="PSUM") as ps1,
        ):
            w_tiles = []
            for k in range(KQ):
                t = wp.tile([128, DK], F32, tag=f"w{k}", name=f"w{k}")
                nc.sync.dma_start(t[:], w[k * 128:(k + 1) * 128, :])
                w_tiles.append(t)

            kt_tiles = []
            vrt = []

            grp = max(1, 8 // LCH)
            groups = list(range(0, KD, grp))
            for g0 in groups:
                dks = range(g0, min(g0 + grp, KD))
                chunks = {}
                for d in dks:
                    for c in range(LCH):
                        chunks[(d, c)] = ps1.tile([128, LCW], F32,
                                                  name=f"ps1_{d}_{c}", tag="ps1")
                for k in range(KQ):
                    qtk = stag.tile([128, L], F32, tag="stag", name=f"qtk{g0}_{k}")
                    nc.sync.dma_start(qtk[:], qT[k * 128:(k + 1) * 128, :])
                    for d in dks:
                        for c in range(LCH):
                            nc.tensor.matmul(
                                chunks[(d, c)][:],
                                w_tiles[k][:, d * 128:(d + 1) * 128],
                                qtk[:, c * LCW:(c + 1) * LCW],
                                start=(k == 0), stop=(k == KQ - 1),
                            )
                if g0 == groups[0]:
                    # kT queues behind group 0's qT stream
                    for k in range(KD):
                        t = ktp.tile([128, S], F32, tag=f"kt{k}", name=f"kt{k}")
                        nc.sync.dma_start(t[:], kT[k * 128:(k + 1) * 128, :])
                        kt_tiles.append(t)
                if g0 == groups[-1]:
                    # V queues behind the last group's qT stream
                    VST = min(DV, 512)
                    for j in range(SJ):
                        vr = vh.tile([128, DV], F32R, tag=f"vr{j}", name=f"vr{j}")
                        for vc in range(DV // VST):
                            vsl = slice(vc * VST, (vc + 1) * VST)
                            st = vstag.tile([128, VST], F32, tag="vstag",
                                            name=f"vstag{j}_{vc}")
                            nc.sync.dma_start(st[:], v[j * 128:(j + 1) * 128, vsl])
                            nc.vector.tensor_copy(vr[:, vsl], st[:])
                        vrt.append(vr)
                for d in dks:
                    for c in range(LCH):
                        nc.vector.tensor_copy(
                            qwt[d][:, c * LCW:(c + 1) * LCW], chunks[(d, c)][:])

        # ---- phase 2/3, software-pipelined per l-tile ----
        # emission order: S(0), sm(0), [S(i), T/AV(i-1), sm(i)]..., T/AV(last)
        # so PE runs T/AV of tile i-1 while tile i's softmax latency resolves.
        with (
            tc.tile_pool(name="ep", bufs=2) as ep,
            tc.tile_pool(name="ptp", bufs=1) as ptp,
            tc.tile_pool(name="op", bufs=2) as op_pool,
            tc.tile_pool(name="scp", bufs=2) as scp,
            tc.tile_pool(name="ps_s", bufs=4, space="PSUM") as ps_s,
            tc.tile_pool(name="ps_t", bufs=2, space="PSUM") as ps_t,
            tc.tile_pool(name="ps_o", bufs=2, space="PSUM") as ps_o,
        ):
            st_chunks = {}
            st_e = {}
            st_rinv = {}

            def emit_S(i):
                lsl = slice(i * 128, (i + 1) * 128)
                schunks = [ps_s.tile([128, CH], F32, name=f"ssc{i}_{c}", tag="ssc")
                           for c in range(NCH)]
                for k in range(KD):
                    for c in range(NCH):
                        nc.tensor.matmul(
                            schunks[c][:],
                            qwt[k][:, lsl],
                            kt_tiles[k][:, c * CH:(c + 1) * CH],
                            start=(k == 0), stop=(k == KD - 1),
                        )
                st_chunks[i] = schunks

            def emit_softmax(i):
                schunks = st_chunks[i]
                scal = scp.tile([128, 8], F32, tag="scal", name=f"scal{i}")
                for c in range(NCH):
                    nc.vector.reduce_max(scal[:, c:c + 1], schunks[c][:],
                                         axis=AX.X, negate=True)
                nm = scal[:, NCH:NCH + 1]
                if NCH == 1:
                    nc.vector.tensor_copy(nm, scal[:, 0:1])
                else:
                    nc.vector.tensor_tensor(nm, scal[:, 0:1], scal[:, 1:2], op=OP.min)
                    for c in range(2, NCH):
                        nc.vector.tensor_tensor(nm, nm, scal[:, c:c + 1], op=OP.min)

                e = ep.tile([128, S], F32, tag="e", name=f"e{i}")
                sums = scp.tile([128, NCH + 2], F32, tag="sums", name=f"sums{i}")
                for c in range(NCH):
                    nc.scalar.activation(e[:, c * CH:(c + 1) * CH], schunks[c][:],
                                         AF.Exp, bias=nm,
                                         accum_out=sums[:, c:c + 1])
                if apply_mask:
                    # multiplicative mask after exp == additive -inf mask
                    # pre-softmax (invalid only for fully-masked rows)
                    nc.vector.tensor_tensor(e[:], e[:], m128[:], op=OP.mult)
                    nc.vector.reduce_sum(sums[:, NCH:NCH + 1], e[:], axis=AX.X)
                    tot = sums[:, NCH:NCH + 1]
                else:
                    tot = sums[:, NCH:NCH + 1]
                    if NCH == 1:
                        tot = sums[:, 0:1]
                    else:
                        nc.vector.tensor_tensor(tot, sums[:, 0:1], sums[:, 1:2],
                                                op=OP.add)
                        for c in range(2, NCH):
                            nc.vector.tensor_tensor(tot, tot, sums[:, c:c + 1],
                                                    op=OP.add)
                rinv = sums[:, NCH + 1:NCH + 2]
                nc.vector.reciprocal(rinv, tot)
                st_e[i] = e
                st_rinv[i] = rinv

            def emit_TAV(i):
                lsl = slice(i * 128, (i + 1) * 128)
                e = st_e[i]
                rinv = st_rinv[i]
                # transpose UNNORMALIZED exp values; fold 1/rowsum into O copy
                ptr = []
                for j in range(SJ):
                    pst = ps_t.tile([128, 128], F32, name=f"pst{i}_{j}", tag="pst")
                    nc.tensor.transpose(pst[:], e[:, j * 128:(j + 1) * 128], ident[:])
                    pr = ptp.tile([128, 128], F32R, tag=f"ptr{j}", name=f"ptr{i}_{j}")
                    nc.vector.tensor_copy(pr[:], pst[:])
                    ptr.append(pr)

                pos = [ps_o.tile([128, OCW], F32, name=f"po{i}_{c}", tag="po")
                       for c in range(OCH)]
                for j in range(SJ):
                    for c in range(OCH):
                        nc.tensor.matmul(pos[c][:], ptr[j][:],
                                         vrt[j][:, c * OCW:(c + 1) * OCW],
                                         start=(j == 0), stop=(j == SJ - 1))
                for c in range(OCH):
                    osl = slice(c * OCW, (c + 1) * OCW)
                    ot = op_pool.tile([128, OCW], F32, tag="ot", name=f"ot{i}_{c}")
                    nc.vector.tensor_scalar_mul(ot[:], pos[c][:], rinv)
                    nc.sync.dma_start(o[lsl, osl], ot[:])
                # normalize P for the score output (off the PE critical path)
                nc.vector.tensor_scalar_mul(e[:], e[:], rinv)
                nc.sync.dma_start(p[lsl, :], e[:])

            emit_S(0)
            emit_softmax(0)
            for i in range(1, LT):
                emit_S(i)
                emit_TAV(i - 1)
                emit_softmax(i)
            emit_TAV(LT - 1)

    nc.compile()
    return nc


_PROGRAM_CACHE = {}


def _get_program(L, S, DQ, DK, DV, apply_mask):
    key = (L, S, DQ, DK, DV, apply_mask)
    if key not in _PROGRAM_CACHE:
        _PROGRAM_CACHE[key] = _build_program(L, S, DQ, DK, DV, apply_mask)
    return _PROGRAM_CACHE[key]


def _run(query, key, value, W, mask, trace=False):
    n, l, dq = query.shape
    _, s, dk = key.shape
    dv = value.shape[2]
    assert n * 2 == N_CORES and l % (2 * 128) == 0
    L = l // 2

    apply_mask = not bool(np.all(mask))
    nc = _get_program(L, s, dq, dk, dv, apply_mask)

    in_maps = []
    for core in range(N_CORES):
        b, h = divmod(core, 2)
        im = {
            "qT": np.ascontiguousarray(
                query[b, h * L:(h + 1) * L, :].T).astype(np.float32, copy=False),
            "w": np.ascontiguousarray(W).astype(np.float32, copy=False),
            "kT": np.ascontiguousarray(key[b].T).astype(np.float32, copy=False),
            "v": np.ascontiguousarray(value[b]).astype(np.float32, copy=False),
        }
        if apply_mask:
            im["maskf"] = np.ascontiguousarray(
                mask[b].astype(np.float32)[None, :])
        in_maps.append(im)

    res = run_bass_kernel_spmd(nc, in_maps, core_ids=list(range(N_CORES)),
                               trace=trace)

    score = np.empty((n, l, s), dtype=np.float32)
    out = np.empty((n, l, dv), dtype=np.float32)
    for core in range(N_CORES):
        b, h = divmod(core, 2)
        score[b, h * L:(h + 1) * L, :] = res.results[core]["p"]
        out[b, h * L:(h + 1) * L, :] = res.results[core]["o"]
    return (out, score), res


def kernel(query, key, value, W, mask):
    (out, score), _ = _run(np.asarray(query), np.asarray(key), np.asarray(value),
                           np.asarray(W), np.asarray(mask))
    return (out, score)


# revision 10
# speedup vs baseline: 1.4995x; 1.1258x over previous
"""BiLinearAttention TRN2 kernel: (out, score) = attention(query, key, value, W, mask).

  score = softmax((query @ W) @ key^T + mask)   [softmax over s]
  out   = score @ value

Sharding: 8 NeuronCores, core = (batch b = core//2, query-half h = core%2).
Each core computes a [1024, 2048] score block + [1024, 1024] output block.

Per-core dataflow:
  All accuracy-critical matmuls use bf16 hi+lo split operands with 3 matmuls
  (hi@hi + lo@hi + hi@lo, fp32 PSUM accumulate) -> ~2^-17 effective operand
  precision at 3 cycles/row (vs fp32's 4).  The P@V matmul uses fp32r
  (hw-rounded 11-bit-mantissa fp32 at full rate).
  phase 1: QWt[dk, l] (lhsT=W hi/lo tiles, rhs=Q^T hi/lo) bf16x3
  per l-tile (software-pipelined): S chunks in PSUM (bf16x3) -> rowmax ->
    exp(bias=-max, rowsum via accum_out) -> PE-transpose unnormalized E
    (fp32, exact) -> cast fp32r -> O = Et@V_f32r scaled by 1/rowsum ->
    P = E/rowsum -> DMA out.
"""
from contextlib import ExitStack

import numpy as np

import concourse.bass as bass
import concourse.mybir as mybir
import concourse.tile as tile
from concourse import bacc
from concourse.bass_utils import run_bass_kernel_spmd
from concourse.masks import make_identity

F32 = mybir.dt.float32
F32R = mybir.dt.float32r
BF16 = mybir.dt.bfloat16
AF = mybir.ActivationFunctionType
AX = mybir.AxisListType
OP = mybir.AluOpType

N_CORES = 8


def _build_program(L, S, DQ, DK, DV, apply_mask, num_devices=N_CORES):
    CH = 512                      # psum chunk width (fp32 bank)
    LT = L // 128                 # l-tiles
    NCH = S // CH                 # s chunks per score row
    KQ = DQ // 128                # contraction tiles for QW
    KD = DK // 128                # dk tiles (partition tiles of QWt)
    SJ = S // 128                 # s-tiles (contraction tiles for AV)
    LCH = max(1, L // CH)         # l chunks in phase 1
    LCW = min(L, CH)
    OCH = max(1, DV // CH)
    OCW = min(DV, CH)

    nc = bacc.Bacc("TRN2", target_bir_lowering=False, debug=False,
                   num_devices=num_devices)

    qT = nc.dram_tensor("qT", [DQ, L], F32, kind="ExternalInput").ap()
    w = nc.dram_tensor("w", [DQ, DK], F32, kind="ExternalInput").ap()
    kT = nc.dram_tensor("kT", [DK, S], F32, kind="ExternalInput").ap()
    v = nc.dram_tensor("v", [S, DV], F32, kind="ExternalInput").ap()
    if apply_mask:
        maskf = nc.dram_tensor("maskf", [1, S], F32, kind="ExternalInput").ap()
    p = nc.dram_tensor("p", [L, S], F32, kind="ExternalOutput").ap()
    o = nc.dram_tensor("o", [L, DV], F32, kind="ExternalOutput").ap()

    with ExitStack() as ctx:
        tc = ctx.enter_context(tile.TileContext(nc))

        const = ctx.enter_context(tc.tile_pool(name="const", bufs=1))
        ktp = ctx.enter_context(tc.tile_pool(name="ktp", bufs=1))
        qwtp = ctx.enter_context(tc.tile_pool(name="qwtp", bufs=1))
        vstag = ctx.enter_context(tc.tile_pool(name="vstag", bufs=2))

        ident = const.tile([128, 128], F32, tag="ident")
        make_identity(nc, ident[:])
        if apply_mask:
            # broadcast mask row to all 128 partitions via K=1 matmul with ones
            mrow = const.tile([1, S], F32, tag="mrow")
            nc.sync.dma_start(mrow[:], maskf[:, :])
            ones = const.tile([1, 128], F32, tag="ones")
            nc.vector.memset(ones[:], 1.0)
            m128 = const.tile([128, S], F32, tag="m128")
            with tc.tile_pool(name="ps_m", bufs=2, space="PSUM") as ps_m:
                for c in range(S // CH):
                    pm = ps_m.tile([128, CH], F32, name=f"pm{c}", tag="pm")
                    nc.tensor.matmul(pm[:], ones[:, :], mrow[:, c * CH:(c + 1) * CH],
                                     start=True, stop=True)
                    nc.vector.tensor_copy(m128[:, c * CH:(c + 1) * CH], pm[:])

        # hi/lo bf16 QWt destination tiles (filled in phase 1)
        qwthi = [qwtp.tile([128, L], BF16, tag=f"qwthi{d}", name=f"qwthi{d}")
                 for d in range(KD)]
        qwtlo = [qwtp.tile([128, L], BF16, tag=f"qwtlo{d}", name=f"qwtlo{d}")
                 for d in range(KD)]
        # hi/lo bf16 K^T tiles (loaded/split during phase 1)
        khi = [ktp.tile([128, S], BF16, tag=f"khi{k}", name=f"khi{k}")
               for k in range(KD)]
        klo = [ktp.tile([128, S], BF16, tag=f"klo{k}", name=f"klo{k}")
               for k in range(KD)]

        def split_from_dram(dst_hi, dst_lo, src_ap, rows, width, jname):
            # stage fp32 rows from DRAM in 512-wide chunks, split to bf16 hi/lo
            CST = min(width, 512)
            for vc in range(width // CST):
                vsl = slice(vc * CST, (vc + 1) * CST)
                st = vstag.tile([128, CST], F32, tag="vstag",
                                name=f"stg_{jname}_{vc}")
                nc.sync.dma_start(st[:], src_ap[rows, vsl])
                nc.vector.tensor_copy(dst_hi[:, vsl], st[:])
                nc.vector.tensor_tensor(dst_lo[:, vsl], st[:], dst_hi[:, vsl],
                                        op=OP.subtract)

        # ---- phase 1: QWt[dk, l] = (Q @ W)^T via bf16x3 ----
        with (
            tc.tile_pool(name="stag", bufs=3) as stag,
            tc.tile_pool(name="qhl", bufs=2) as qhl,
            tc.tile_pool(name="wp", bufs=1) as wp,
            tc.tile_pool(name="ps1", bufs=8, space="PSUM") as ps1,
        ):
            whi = [wp.tile([128, DK], BF16, tag=f"whi{k}", name=f"whi{k}")
                   for k in range(KQ)]
            wlo = [wp.tile([128, DK], BF16, tag=f"wlo{k}", name=f"wlo{k}")
                   for k in range(KQ)]

            grp = max(1, 8 // LCH)
            groups = list(range(0, KD, grp))
            for g0 in groups:
                dks = range(g0, min(g0 + grp, KD))
                chunks = {}
                for d in dks:
                    for c in range(LCH):
                        chunks[(d, c)] = ps1.tile([128, LCW], F32,
                                                  name=f"ps1_{d}_{c}", tag="ps1")
                for k in range(KQ):
                    if g0 == groups[0]:
                        # W arrives interleaved with the qT stream
                        wst = stag.tile([128, DK], F32, tag="stag",
                                        name=f"wst{k}")
                        nc.sync.dma_start(wst[:], w[k * 128:(k + 1) * 128, :])
                        nc.vector.tensor_copy(whi[k][:], wst[:])
                        nc.vector.tensor_tensor(wlo[k][:], wst[:], whi[k][:],
                                                op=OP.subtract)
                    qst = stag.tile([128, L], F32, tag="stag", name=f"qst{g0}_{k}")
                    nc.sync.dma_start(qst[:], qT[k * 128:(k + 1) * 128, :])
                    qh = qhl.tile([128, L], BF16, tag="qh", name=f"qh{g0}_{k}")
                    ql = qhl.tile([128, L], BF16, tag="ql", name=f"ql{g0}_{k}")
                    nc.vector.tensor_copy(qh[:], qst[:])
                    nc.vector.tensor_tensor(ql[:], qst[:], qh[:], op=OP.subtract)
                    for d in dks:
                        dsl = slice(d * 128, (d + 1) * 128)
                        for c in range(LCH):
                            csl = slice(c * LCW, (c + 1) * LCW)
                            ps = chunks[(d, c)][:]
                            nc.tensor.matmul(ps, whi[k][:, dsl], qh[:, csl],
                                             start=(k == 0), stop=False)
                            nc.tensor.matmul(ps, wlo[k][:, dsl], qh[:, csl],
                                             start=False, stop=False)
                            nc.tensor.matmul(ps, whi[k][:, dsl], ql[:, csl],
                                             start=False, stop=(k == KQ - 1))
                    if g0 == groups[0] and k == KQ - 1:
                        # kT loads queue behind group 0's inputs; split to
                        # bf16 hi/lo while group 1 computes
                        for kk in range(KD):
                            split_from_dram(khi[kk], klo[kk],
                                            kT, slice(kk * 128, (kk + 1) * 128),
                                            S, f"kt{kk}")
                for d in dks:
                    for c in range(LCH):
                        csl = slice(c * LCW, (c + 1) * LCW)
                        nc.vector.tensor_copy(qwthi[d][:, csl], chunks[(d, c)][:])
                        nc.vector.tensor_tensor(qwtlo[d][:, csl], chunks[(d, c)][:],
                                                qwthi[d][:, csl], op=OP.subtract)

        # ---- V load as f32r (phase-1 pools closed; space freed) ----
        vh = ctx.enter_context(tc.tile_pool(name="vh", bufs=1))
        vrt = []
        VST = min(DV, 512)
        for j in range(SJ):
            vr = vh.tile([128, DV], F32R, tag=f"vr{j}", name=f"vr{j}")
            for vc in range(DV // VST):
                vsl = slice(vc * VST, (vc + 1) * VST)
                st = vstag.tile([128, VST], F32, tag="vstag",
                                name=f"vstag{j}_{vc}")
                nc.sync.dma_start(st[:], v[j * 128:(j + 1) * 128, vsl])
                nc.vector.tensor_copy(vr[:, vsl], st[:])
            vrt.append(vr)

        # ---- phase 2/3, software-pipelined per l-tile ----
        # emission order: S(0), sm(0), [S(i), T/AV(i-1), sm(i)]..., T/AV(last)
        # so PE runs T/AV of tile i-1 while tile i's softmax latency resolves.
        with (
            tc.tile_pool(name="ep", bufs=2) as ep,
            tc.tile_pool(name="ptp", bufs=1) as ptp,
            tc.tile_pool(name="op", bufs=2) as op_pool,
            tc.tile_pool(name="scp", bufs=2) as scp,
            tc.tile_pool(name="ps_s", bufs=4, space="PSUM") as ps_s,
            tc.tile_pool(name="ps_t", bufs=2, space="PSUM") as ps_t,
            tc.tile_pool(name="ps_o", bufs=2, space="PSUM") as ps_o,
        ):
            st_chunks = {}
            st_e = {}
            st_rinv = {}

            def emit_S(i):
                lsl = slice(i * 128, (i + 1) * 128)
                schunks = [ps_s.tile([128, CH], F32, name=f"ssc{i}_{c}", tag="ssc")
                           for c in range(NCH)]
                for k in range(KD):
                    for c in range(NCH):
                        csl = slice(c * CH, (c + 1) * CH)
                        nc.tensor.matmul(schunks[c][:], qwthi[k][:, lsl],
                                         khi[k][:, csl],
                                         start=(k == 0), stop=False)
                        nc.tensor.matmul(schunks[c][:], qwtlo[k][:, lsl],
                                         khi[k][:, csl],
                                         start=False, stop=False)
                        nc.tensor.matmul(schunks[c][:], qwthi[k][:, lsl],
                                         klo[k][:, csl],
                                         start=False, stop=(k == KD - 1))
                st_chunks[i] = schunks

            def emit_softmax(i):
                schunks = st_chunks[i]
                scal = scp.tile([128, 8], F32, tag="scal", name=f"scal{i}")
                for c in range(NCH):
                    nc.vector.reduce_max(scal[:, c:c + 1], schunks[c][:],
                                         axis=AX.X, negate=True)
                nm = scal[:, NCH:NCH + 1]
                if NCH == 1:
                    nc.vector.tensor_copy(nm, scal[:, 0:1])
                else:
                    nc.vector.tensor_tensor(nm, scal[:, 0:1], scal[:, 1:2], op=OP.min)
                    for c in range(2, NCH):
                        nc.vector.tensor_tensor(nm, nm, scal[:, c:c + 1], op=OP.min)

                e = ep.tile([128, S], F32, tag="e", name=f"e{i}")
                sums = scp.tile([128, NCH + 2], F32, tag="sums", name=f"sums{i}")
                for c in range(NCH):
                    nc.scalar.activation(e[:, c * CH:(c + 1) * CH], schunks[c][:],
                                         AF.Exp, bias=nm,
                                         accum_out=sums[:, c:c + 1])
                if apply_mask:
                    # multiplicative mask after exp == additive -inf mask
                    # pre-softmax (invalid only for fully-masked rows)
                    nc.vector.tensor_tensor(e[:], e[:], m128[:], op=OP.mult)
                    nc.vector.reduce_sum(sums[:, NCH:NCH + 1], e[:], axis=AX.X)
                    tot = sums[:, NCH:NCH + 1]
                else:
                    tot = sums[:, NCH:NCH + 1]
                    if NCH == 1:
                        tot = sums[:, 0:1]
                    else:
                        nc.vector.tensor_tensor(tot, sums[:, 0:1], sums[:, 1:2],
                                                op=OP.add)
                        for c in range(2, NCH):
                            nc.vector.tensor_tensor(tot, tot, sums[:, c:c + 1],
                                                    op=OP.add)
                rinv = sums[:, NCH + 1:NCH + 2]
                nc.vector.reciprocal(rinv, tot)
                st_e[i] = e
                st_rinv[i] = rinv

            def emit_TAV(i):
                lsl = slice(i * 128, (i + 1) * 128)
                e = st_e[i]
                rinv = st_rinv[i]
                # transpose UNNORMALIZED exp values; fold 1/rowsum into O copy
                ptr = []
                for j in range(SJ):
                    pst = ps_t.tile([128, 128], F32, name=f"pst{i}_{j}", tag="pst")
                    nc.tensor.transpose(pst[:], e[:, j * 128:(j + 1) * 128], ident[:])
                    pr = ptp.tile([128, 128], F32R, tag=f"ptr{j}", name=f"ptr{i}_{j}")
                    nc.vector.tensor_copy(pr[:], pst[:])
                    ptr.append(pr)

                pos = [ps_o.tile([128, OCW], F32, name=f"po{i}_{c}", tag="po")
                       for c in range(OCH)]
                for j in range(SJ):
                    for c in range(OCH):
                        nc.tensor.matmul(pos[c][:], ptr[j][:],
                                         vrt[j][:, c * OCW:(c + 1) * OCW],
                                         start=(j == 0), stop=(j == SJ - 1))
                for c in range(OCH):
                    osl = slice(c * OCW, (c + 1) * OCW)
                    ot = op_pool.tile([128, OCW], F32, tag="ot", name=f"ot{i}_{c}")
                    nc.vector.tensor_scalar_mul(ot[:], pos[c][:], rinv)
                    nc.sync.dma_start(o[lsl, osl], ot[:])
                # normalize P for the score output (off the PE critical path)
                nc.vector.tensor_scalar_mul(e[:], e[:], rinv)
                nc.sync.dma_start(p[lsl, :], e[:])

            emit_S(0)
            emit_softmax(0)
            for i in range(1, LT):
                emit_S(i)
                emit_TAV(i - 1)
                emit_softmax(i)
            emit_TAV(LT - 1)

    nc.compile()
    return nc


_PROGRAM_CACHE = {}


def _get_program(L, S, DQ, DK, DV, apply_mask):
    key = (L, S, DQ, DK, DV, apply_mask)
    if key not in _PROGRAM_CACHE:
        _PROGRAM_CACHE[key] = _build_program(L, S, DQ, DK, DV, apply_mask)
    return _PROGRAM_CACHE[key]


def _run(query, key, value, W, mask, trace=False):
    n, l, dq = query.shape
    _, s, dk = key.shape
    dv = value.shape[2]
    assert n * 2 == N_CORES and l % (2 * 128) == 0
    L = l // 2

    apply_mask = not bool(np.all(mask))
    nc = _get_program(L, s, dq, dk, dv, apply_mask)

    in_maps = []
    for core in range(N_CORES):
        b, h = divmod(core, 2)
        im = {
            "qT": np.ascontiguousarray(
                query[b, h * L:(h + 1) * L, :].T).astype(np.float32, copy=False),
            "w": np.ascontiguousarray(W).astype(np.float32, copy=False),
            "kT": np.ascontiguousarray(key[b].T).astype(np.float32, copy=False),
            "v": np.ascontiguousarray(value[b]).astype(np.float32, copy=False),
        }
        if apply_mask:
            im["maskf"] = np.ascontiguousarray(
                mask[b].astype(np.float32)[None, :])
        in_maps.append(im)

    res = run_bass_kernel_spmd(nc, in_maps, core_ids=list(range(N_CORES)),
                               trace=trace)

    score = np.empty((n, l, s), dtype=np.float32)
    out = np.empty((n, l, dv), dtype=np.float32)
    for core in range(N_CORES):
        b, h = divmod(core, 2)
        score[b, h * L:(h + 1) * L, :] = res.results[core]["p"]
        out[b, h * L:(h + 1) * L, :] = res.results[core]["o"]
    return (out, score), res


def kernel(query, key, value, W, mask):
    (out, score), _ = _run(np.asarray(query), np.asarray(key), np.asarray(value),
                           np.asarray(W), np.asarray(mask))
    return (out, score)


# revision 11
# speedup vs baseline: 1.5632x; 1.0425x over previous
"""BiLinearAttention TRN2 kernel: (out, score) = attention(query, key, value, W, mask).

  score = softmax((query @ W) @ key^T + mask)   [softmax over s]
  out   = score @ value

Sharding: 8 NeuronCores, core = (batch b = core//2, query-half h = core%2).
Each core computes a [1024, 2048] score block + [1024, 1024] output block.

Per-core dataflow:
  All accuracy-critical matmuls use bf16 hi+lo split operands with 3 matmuls
  (hi@hi + lo@hi + hi@lo, fp32 PSUM accumulate) -> ~2^-17 effective operand
  precision at 3 cycles/row (vs fp32's 4).  The P@V matmul uses fp32r
  (hw-rounded 11-bit-mantissa fp32 at full rate).
  phase 1: QWt[dk, l] (lhsT=W hi/lo tiles, rhs=Q^T hi/lo) bf16x3
  per l-tile (software-pipelined): S chunks in PSUM (bf16x3) -> rowmax ->
    exp(bias=-max, rowsum via accum_out) -> PE-transpose unnormalized E
    (fp32, exact) -> cast fp32r -> O = Et@V_f32r scaled by 1/rowsum ->
    P = E/rowsum -> DMA out.
"""
from contextlib import ExitStack

import numpy as np

import concourse.bass as bass
import concourse.mybir as mybir
import concourse.tile as tile
from concourse import bacc
from concourse.bass_utils import run_bass_kernel_spmd
from concourse.masks import make_identity

F32 = mybir.dt.float32
F32R = mybir.dt.float32r
BF16 = mybir.dt.bfloat16
AF = mybir.ActivationFunctionType
AX = mybir.AxisListType
OP = mybir.AluOpType

N_CORES = 8


def _build_program(L, S, DQ, DK, DV, apply_mask, num_devices=N_CORES):
    CH = 512                      # psum chunk width (fp32 bank)
    LT = L // 128                 # l-tiles
    NCH = S // CH                 # s chunks per score row
    KQ = DQ // 128                # contraction tiles for QW
    KD = DK // 128                # dk tiles (partition tiles of QWt)
    SJ = S // 128                 # s-tiles (contraction tiles for AV)
    LCH = max(1, L // CH)         # l chunks in phase 1
    LCW = min(L, CH)
    OCH = max(1, DV // CH)
    OCW = min(DV, CH)

    nc = bacc.Bacc("TRN2", target_bir_lowering=False, debug=False,
                   num_devices=num_devices)

    qT = nc.dram_tensor("qT", [DQ, L], F32, kind="ExternalInput").ap()
    w = nc.dram_tensor("w", [DQ, DK], F32, kind="ExternalInput").ap()
    kT = nc.dram_tensor("kT", [DK, S], F32, kind="ExternalInput").ap()
    v = nc.dram_tensor("v", [S, DV], F32, kind="ExternalInput").ap()
    if apply_mask:
        maskf = nc.dram_tensor("maskf", [1, S], F32, kind="ExternalInput").ap()
    p = nc.dram_tensor("p", [L, S], F32, kind="ExternalOutput").ap()
    o = nc.dram_tensor("o", [L, DV], F32, kind="ExternalOutput").ap()

    with ExitStack() as ctx:
        tc = ctx.enter_context(tile.TileContext(nc))

        const = ctx.enter_context(tc.tile_pool(name="const", bufs=1))
        ktp = ctx.enter_context(tc.tile_pool(name="ktp", bufs=1))
        qwtp = ctx.enter_context(tc.tile_pool(name="qwtp", bufs=1))
        vstag = ctx.enter_context(tc.tile_pool(name="vstag", bufs=2))

        ident = const.tile([128, 128], F32, tag="ident")
        make_identity(nc, ident[:])
        if apply_mask:
            # broadcast mask row to all 128 partitions via K=1 matmul with ones
            mrow = const.tile([1, S], F32, tag="mrow")
            nc.sync.dma_start(mrow[:], maskf[:, :])
            ones = const.tile([1, 128], F32, tag="ones")
            nc.vector.memset(ones[:], 1.0)
            m128 = const.tile([128, S], F32, tag="m128")
            with tc.tile_pool(name="ps_m", bufs=2, space="PSUM") as ps_m:
                for c in range(S // CH):
                    pm = ps_m.tile([128, CH], F32, name=f"pm{c}", tag="pm")
                    nc.tensor.matmul(pm[:], ones[:, :], mrow[:, c * CH:(c + 1) * CH],
                                     start=True, stop=True)
                    nc.vector.tensor_copy(m128[:, c * CH:(c + 1) * CH], pm[:])

        # hi/lo bf16 QWt destination tiles (filled in phase 1)
        qwthi = [qwtp.tile([128, L], BF16, tag=f"qwthi{d}", name=f"qwthi{d}")
                 for d in range(KD)]
        qwtlo = [qwtp.tile([128, L], BF16, tag=f"qwtlo{d}", name=f"qwtlo{d}")
                 for d in range(KD)]
        # hi/lo bf16 K^T tiles (loaded/split during phase 1)
        khi = [ktp.tile([128, S], BF16, tag=f"khi{k}", name=f"khi{k}")
               for k in range(KD)]
        klo = [ktp.tile([128, S], BF16, tag=f"klo{k}", name=f"klo{k}")
               for k in range(KD)]

        def split_from_dram(dst_hi, dst_lo, src_ap, rows, width, jname):
            # stage fp32 rows from DRAM in 512-wide chunks, split to bf16 hi/lo
            CST = min(width, 512)
            for vc in range(width // CST):
                vsl = slice(vc * CST, (vc + 1) * CST)
                st = vstag.tile([128, CST], F32, tag="vstag",
                                name=f"stg_{jname}_{vc}")
                nc.sync.dma_start(st[:], src_ap[rows, vsl])
                nc.scalar.activation(dst_hi[:, vsl], st[:], AF.Copy)
                nc.vector.tensor_tensor(dst_lo[:, vsl], st[:], dst_hi[:, vsl],
                                        op=OP.subtract)

        # ---- phase 1: QWt[dk, l] = (Q @ W)^T via bf16x3 ----
        with (
            tc.tile_pool(name="stag", bufs=3) as stag,
            tc.tile_pool(name="qhl", bufs=2) as qhl,
            tc.tile_pool(name="wp", bufs=1) as wp,
            tc.tile_pool(name="ps1", bufs=8, space="PSUM") as ps1,
        ):
            whi = [wp.tile([128, DK], BF16, tag=f"whi{k}", name=f"whi{k}")
                   for k in range(KQ)]
            wlo = [wp.tile([128, DK], BF16, tag=f"wlo{k}", name=f"wlo{k}")
                   for k in range(KQ)]

            grp = max(1, 8 // LCH)
            groups = list(range(0, KD, grp))
            for g0 in groups:
                dks = range(g0, min(g0 + grp, KD))
                chunks = {}
                for d in dks:
                    for c in range(LCH):
                        chunks[(d, c)] = ps1.tile([128, LCW], F32,
                                                  name=f"ps1_{d}_{c}", tag="ps1")
                for k in range(KQ):
                    if g0 == groups[0]:
                        # W arrives interleaved with the qT stream
                        wst = stag.tile([128, DK], F32, tag="stag",
                                        name=f"wst{k}")
                        nc.sync.dma_start(wst[:], w[k * 128:(k + 1) * 128, :])
                        nc.vector.tensor_copy(whi[k][:], wst[:])
                        nc.vector.tensor_tensor(wlo[k][:], wst[:], whi[k][:],
                                                op=OP.subtract)
                    qst = stag.tile([128, L], F32, tag="stag", name=f"qst{g0}_{k}")
                    nc.sync.dma_start(qst[:], qT[k * 128:(k + 1) * 128, :])
                    qh = qhl.tile([128, L], BF16, tag="qh", name=f"qh{g0}_{k}")
                    ql = qhl.tile([128, L], BF16, tag="ql", name=f"ql{g0}_{k}")
                    nc.vector.tensor_copy(qh[:], qst[:])
                    nc.vector.tensor_tensor(ql[:], qst[:], qh[:], op=OP.subtract)
                    for d in dks:
                        dsl = slice(d * 128, (d + 1) * 128)
                        for c in range(LCH):
                            csl = slice(c * LCW, (c + 1) * LCW)
                            ps = chunks[(d, c)][:]
                            nc.tensor.matmul(ps, whi[k][:, dsl], qh[:, csl],
                                             start=(k == 0), stop=False)
                            nc.tensor.matmul(ps, wlo[k][:, dsl], qh[:, csl],
                                             start=False, stop=False)
                            nc.tensor.matmul(ps, whi[k][:, dsl], ql[:, csl],
                                             start=False, stop=(k == KQ - 1))
                    if g0 == groups[0] and k == KQ - 1:
                        # kT loads queue behind group 0's inputs; split to
                        # bf16 hi/lo while group 1 computes
                        for kk in range(KD):
                            split_from_dram(khi[kk], klo[kk],
                                            kT, slice(kk * 128, (kk + 1) * 128),
                                            S, f"kt{kk}")
                for d in dks:
                    for c in range(LCH):
                        csl = slice(c * LCW, (c + 1) * LCW)
                        nc.scalar.activation(qwthi[d][:, csl], chunks[(d, c)][:],
                                             AF.Copy)
                        nc.vector.tensor_tensor(qwtlo[d][:, csl], chunks[(d, c)][:],
                                                qwthi[d][:, csl], op=OP.subtract)

        # ---- V load as f32r (phase-1 pools closed; space freed) ----
        vh = ctx.enter_context(tc.tile_pool(name="vh", bufs=1))
        vrt = []
        VST = min(DV, 512)
        for j in range(SJ):
            vr = vh.tile([128, DV], F32R, tag=f"vr{j}", name=f"vr{j}")
            for vc in range(DV // VST):
                vsl = slice(vc * VST, (vc + 1) * VST)
                st = vstag.tile([128, VST], F32, tag="vstag",
                                name=f"vstag{j}_{vc}")
                nc.sync.dma_start(st[:], v[j * 128:(j + 1) * 128, vsl])
                nc.scalar.activation(vr[:, vsl], st[:], AF.Copy)
            vrt.append(vr)

        # ---- phase 2/3, software-pipelined per l-tile ----
        # emission order: S(0), sm(0), [S(i), T/AV(i-1), sm(i)]..., T/AV(last)
        # so PE runs T/AV of tile i-1 while tile i's softmax latency resolves.
        with (
            tc.tile_pool(name="ep", bufs=2) as ep,
            tc.tile_pool(name="ptp", bufs=1) as ptp,
            tc.tile_pool(name="op", bufs=2) as op_pool,
            tc.tile_pool(name="scp", bufs=2) as scp,
            tc.tile_pool(name="ps_s", bufs=4, space="PSUM") as ps_s,
            tc.tile_pool(name="ps_t", bufs=2, space="PSUM") as ps_t,
            tc.tile_pool(name="ps_o", bufs=2, space="PSUM") as ps_o,
        ):
            st_chunks = {}
            st_e = {}
            st_rinv = {}

            def emit_S(i):
                lsl = slice(i * 128, (i + 1) * 128)
                schunks = [ps_s.tile([128, CH], F32, name=f"ssc{i}_{c}", tag="ssc")
                           for c in range(NCH)]
                for k in range(KD):
                    for c in range(NCH):
                        csl = slice(c * CH, (c + 1) * CH)
                        nc.tensor.matmul(schunks[c][:], qwthi[k][:, lsl],
                                         khi[k][:, csl],
                                         start=(k == 0), stop=False)
                        nc.tensor.matmul(schunks[c][:], qwtlo[k][:, lsl],
                                         khi[k][:, csl],
                                         start=False, stop=False)
                        nc.tensor.matmul(schunks[c][:], qwthi[k][:, lsl],
                                         klo[k][:, csl],
                                         start=False, stop=(k == KD - 1))
                st_chunks[i] = schunks

            def emit_softmax(i):
                schunks = st_chunks[i]
                scal = scp.tile([128, 8], F32, tag="scal", name=f"scal{i}")
                for c in range(NCH):
                    nc.vector.reduce_max(scal[:, c:c + 1], schunks[c][:],
                                         axis=AX.X, negate=True)
                nm = scal[:, NCH:NCH + 1]
                if NCH == 1:
                    nc.vector.tensor_copy(nm, scal[:, 0:1])
                else:
                    nc.vector.tensor_tensor(nm, scal[:, 0:1], scal[:, 1:2], op=OP.min)
                    for c in range(2, NCH):
                        nc.vector.tensor_tensor(nm, nm, scal[:, c:c + 1], op=OP.min)

                e = ep.tile([128, S], F32, tag="e", name=f"e{i}")
                sums = scp.tile([128, NCH + 2], F32, tag="sums", name=f"sums{i}")
                for c in range(NCH):
                    nc.scalar.activation(e[:, c * CH:(c + 1) * CH], schunks[c][:],
                                         AF.Exp, bias=nm,
                                         accum_out=sums[:, c:c + 1])
                if apply_mask:
                    # multiplicative mask after exp == additive -inf mask
                    # pre-softmax (invalid only for fully-masked rows)
                    nc.vector.tensor_tensor(e[:], e[:], m128[:], op=OP.mult)
                    nc.vector.reduce_sum(sums[:, NCH:NCH + 1], e[:], axis=AX.X)
                    tot = sums[:, NCH:NCH + 1]
                else:
                    tot = sums[:, NCH:NCH + 1]
                    if NCH == 1:
                        tot = sums[:, 0:1]
                    else:
                        nc.vector.tensor_tensor(tot, sums[:, 0:1], sums[:, 1:2],
                                                op=OP.add)
                        for c in range(2, NCH):
                            nc.vector.tensor_tensor(tot, tot, sums[:, c:c + 1],
                                                    op=OP.add)
                rinv = sums[:, NCH + 1:NCH + 2]
                nc.vector.reciprocal(rinv, tot)
                st_e[i] = e
                st_rinv[i] = rinv

            def emit_TAV(i, last=False):
                lsl = slice(i * 128, (i + 1) * 128)
                e = st_e[i]
                rinv = st_rinv[i]
                if last:
                    # final tile: normalize first (nothing left to overlap),
                    # so the tail is transpose->AV->copy only
                    nc.scalar.activation(e[:], e[:], AF.Copy, scale=rinv)
                    nc.sync.dma_start(p[lsl, :], e[:])
                # transpose UNNORMALIZED exp values; fold 1/rowsum into O copy
                ptr = []
                for j in range(SJ):
                    pst = ps_t.tile([128, 128], F32, name=f"pst{i}_{j}", tag="pst")
                    nc.tensor.transpose(pst[:], e[:, j * 128:(j + 1) * 128], ident[:])
                    pr = ptp.tile([128, 128], F32R, tag=f"ptr{j}", name=f"ptr{i}_{j}")
                    nc.vector.tensor_copy(pr[:], pst[:])
                    ptr.append(pr)

                pos = [ps_o.tile([128, OCW], F32, name=f"po{i}_{c}", tag="po")
                       for c in range(OCH)]
                for j in range(SJ):
                    for c in range(OCH):
                        nc.tensor.matmul(pos[c][:], ptr[j][:],
                                         vrt[j][:, c * OCW:(c + 1) * OCW],
                                         start=(j == 0), stop=(j == SJ - 1))
                for c in range(OCH):
                    osl = slice(c * OCW, (c + 1) * OCW)
                    ot = op_pool.tile([128, OCW], F32, tag="ot", name=f"ot{i}_{c}")
                    if last:
                        nc.vector.tensor_copy(ot[:], pos[c][:])
                    else:
                        nc.vector.tensor_scalar_mul(ot[:], pos[c][:], rinv)
                    nc.sync.dma_start(o[lsl, osl], ot[:])
                if not last:
                    # normalize P for the score output (off the PE critical path)
                    nc.scalar.activation(e[:], e[:], AF.Copy, scale=rinv)
                    nc.sync.dma_start(p[lsl, :], e[:])

            emit_S(0)
            emit_softmax(0)
            for i in range(1, LT):
                emit_S(i)
                emit_TAV(i - 1)
                emit_softmax(i)
            emit_TAV(LT - 1, last=True)

    nc.compile()
    return nc


_PROGRAM_CACHE = {}


def _get_program(L, S, DQ, DK, DV, apply_mask):
    key = (L, S, DQ, DK, DV, apply_mask)
    if key not in _PROGRAM_CACHE:
        _PROGRAM_CACHE[key] = _build_program(L, S, DQ, DK, DV, apply_mask)
    return _PROGRAM_CACHE[key]


def _run(query, key, value, W, mask, trace=False):
    n, l, dq = query.shape
    _, s, dk = key.shape
    dv = value.shape[2]
    assert n * 2 == N_CORES and l % (2 * 128) == 0
    L = l // 2

    apply_mask = not bool(np.all(mask))
    nc = _get_program(L, s, dq, dk, dv, apply_mask)

    in_maps = []
    for core in range(N_CORES):
        b, h = divmod(core, 2)
        im = {
            "qT": np.ascontiguousarray(
                query[b, h * L:(h + 1) * L, :].T).astype(np.float32, copy=False),
            "w": np.ascontiguousarray(W).astype(np.float32, copy=False),
            "kT": np.ascontiguousarray(key[b].T).astype(np.float32, copy=False),
            "v": np.ascontiguousarray(value[b]).astype(np.float32, copy=False),
        }
        if apply_mask:
            im["maskf"] = np.ascontiguousarray(
                mask[b].astype(np.float32)[None, :])
        in_maps.append(im)

    res = run_bass_kernel_spmd(nc, in_maps, core_ids=list(range(N_CORES)),
                               trace=trace)

    score = np.empty((n, l, s), dtype=np.float32)
    out = np.empty((n, l, dv), dtype=np.float32)
    for core in range(N_CORES):
        b, h = divmod(core, 2)
        score[b, h * L:(h + 1) * L, :] = res.results[core]["p"]
        out[b, h * L:(h + 1) * L, :] = res.results[core]["o"]
    return (out, score), res


def kernel(query, key, value, W, mask):
    (out, score), _ = _run(np.asarray(query), np.asarray(key), np.asarray(value),
                           np.asarray(W), np.asarray(mask))
    return (out, score)


# revision 12
# speedup vs baseline: 1.6261x; 1.0403x over previous
"""BiLinearAttention TRN2 kernel: (out, score) = attention(query, key, value, W, mask).

  score = softmax((query @ W) @ key^T + mask)   [softmax over s]
  out   = score @ value

Sharding: 8 NeuronCores, core = (batch b = core//2, query-half h = core%2).
Each core computes a [1024, 2048] score block + [1024, 1024] output block.

Per-core dataflow:
  All accuracy-critical matmuls use bf16 hi+lo split operands with 3 matmuls
  (hi@hi + lo@hi + hi@lo, fp32 PSUM accumulate) -> ~2^-17 effective operand
  precision at 3 cycles/row (vs fp32's 4).  The P@V matmul uses fp32r
  (hw-rounded 11-bit-mantissa fp32 at full rate).
  phase 1: QWt[dk, l] (lhsT=W hi/lo tiles, rhs=Q^T hi/lo) bf16x3
  per l-tile (software-pipelined): S chunks in PSUM (bf16x3) -> rowmax ->
    exp(bias=-max, rowsum via accum_out) -> PE-transpose unnormalized E
    (fp32, exact) -> cast fp32r -> O = Et@V_f32r scaled by 1/rowsum ->
    P = E/rowsum -> DMA out.
"""
from contextlib import ExitStack

import numpy as np

import concourse.bass as bass
import concourse.mybir as mybir
import concourse.tile as tile
from concourse import bacc
from concourse.bass_utils import run_bass_kernel_spmd
from concourse.masks import make_identity

F32 = mybir.dt.float32
F32R = mybir.dt.float32r
BF16 = mybir.dt.bfloat16
AF = mybir.ActivationFunctionType
AX = mybir.AxisListType
OP = mybir.AluOpType

N_CORES = 8


def _build_program(L, S, DQ, DK, DV, apply_mask, num_devices=N_CORES):
    CH = 512                      # psum chunk width (fp32 bank)
    LT = L // 128                 # l-tiles
    NCH = S // CH                 # s chunks per score row
    KQ = DQ // 128                # contraction tiles for QW
    KD = DK // 128                # dk tiles (partition tiles of QWt)
    SJ = S // 128                 # s-tiles (contraction tiles for AV)
    LCH = max(1, L // CH)         # l chunks in phase 1
    LCW = min(L, CH)
    OCH = max(1, DV // CH)
    OCW = min(DV, CH)

    nc = bacc.Bacc("TRN2", target_bir_lowering=False, debug=False,
                   num_devices=num_devices)

    qT = nc.dram_tensor("qT", [DQ, L], F32, kind="ExternalInput").ap()
    w = nc.dram_tensor("w", [DQ, DK], F32, kind="ExternalInput").ap()
    kT = nc.dram_tensor("kT", [DK, S], F32, kind="ExternalInput").ap()
    v = nc.dram_tensor("v", [S, DV], F32, kind="ExternalInput").ap()
    if apply_mask:
        maskf = nc.dram_tensor("maskf", [1, S], F32, kind="ExternalInput").ap()
    p = nc.dram_tensor("p", [L, S], F32, kind="ExternalOutput").ap()
    o = nc.dram_tensor("o", [L, DV], F32, kind="ExternalOutput").ap()

    with ExitStack() as ctx:
        tc = ctx.enter_context(tile.TileContext(nc))

        const = ctx.enter_context(tc.tile_pool(name="const", bufs=1))
        ktp = ctx.enter_context(tc.tile_pool(name="ktp", bufs=1))
        qwtp = ctx.enter_context(tc.tile_pool(name="qwtp", bufs=1))
        vstag = ctx.enter_context(tc.tile_pool(name="vstag", bufs=2))

        ident = const.tile([128, 128], F32, tag="ident")
        make_identity(nc, ident[:])
        if apply_mask:
            # broadcast mask row to all 128 partitions via K=1 matmul with ones
            mrow = const.tile([1, S], F32, tag="mrow")
            nc.sync.dma_start(mrow[:], maskf[:, :])
            ones = const.tile([1, 128], F32, tag="ones")
            nc.vector.memset(ones[:], 1.0)
            m128 = const.tile([128, S], F32, tag="m128")
            with tc.tile_pool(name="ps_m", bufs=2, space="PSUM") as ps_m:
                for c in range(S // CH):
                    pm = ps_m.tile([128, CH], F32, name=f"pm{c}", tag="pm")
                    nc.tensor.matmul(pm[:], ones[:, :], mrow[:, c * CH:(c + 1) * CH],
                                     start=True, stop=True)
                    nc.vector.tensor_copy(m128[:, c * CH:(c + 1) * CH], pm[:])

        # hi/lo bf16 QWt destination tiles (filled in phase 1)
        qwthi = [qwtp.tile([128, L], BF16, tag=f"qwthi{d}", name=f"qwthi{d}")
                 for d in range(KD)]
        qwtlo = [qwtp.tile([128, L], BF16, tag=f"qwtlo{d}", name=f"qwtlo{d}")
                 for d in range(KD)]
        # hi/lo bf16 K^T tiles (loaded/split during phase 1)
        khi = [ktp.tile([128, S], BF16, tag=f"khi{k}", name=f"khi{k}")
               for k in range(KD)]
        klo = [ktp.tile([128, S], BF16, tag=f"klo{k}", name=f"klo{k}")
               for k in range(KD)]

        def split_from_dram(dst_hi, dst_lo, src_ap, rows, width, jname):
            # stage fp32 rows from DRAM in 512-wide chunks, split to bf16 hi/lo
            CST = min(width, 512)
            for vc in range(width // CST):
                vsl = slice(vc * CST, (vc + 1) * CST)
                st = vstag.tile([128, CST], F32, tag="vstag",
                                name=f"stg_{jname}_{vc}")
                nc.sync.dma_start(st[:], src_ap[rows, vsl])
                nc.scalar.activation(dst_hi[:, vsl], st[:], AF.Copy)
                nc.vector.tensor_tensor(dst_lo[:, vsl], st[:], dst_hi[:, vsl],
                                        op=OP.subtract)

        # ---- phase 1: QWt[dk, l] = (Q @ W)^T via bf16x3 ----
        with (
            tc.tile_pool(name="stag", bufs=3) as stag,
            tc.tile_pool(name="qhl", bufs=1) as qhl,
            tc.tile_pool(name="wp", bufs=1) as wp,
            tc.tile_pool(name="ps1", bufs=8, space="PSUM") as ps1,
        ):
            whi = [wp.tile([128, DK], BF16, tag=f"whi{k}", name=f"whi{k}")
                   for k in range(KQ)]
            wlo = [wp.tile([128, DK], BF16, tag=f"wlo{k}", name=f"wlo{k}")
                   for k in range(KQ)]

            qhs, qls = [], []
            grp = max(1, 8 // LCH)
            groups = list(range(0, KD, grp))
            for g0 in groups:
                dks = range(g0, min(g0 + grp, KD))
                chunks = {}
                for d in dks:
                    for c in range(LCH):
                        chunks[(d, c)] = ps1.tile([128, LCW], F32,
                                                  name=f"ps1_{d}_{c}", tag="ps1")
                for k in range(KQ):
                    if g0 == groups[0]:
                        # W arrives interleaved with the qT stream; qT hi/lo
                        # kept resident so later groups reuse them (single pass)
                        wst = stag.tile([128, DK], F32, tag="stag",
                                        name=f"wst{k}")
                        nc.sync.dma_start(wst[:], w[k * 128:(k + 1) * 128, :])
                        nc.vector.tensor_copy(whi[k][:], wst[:])
                        nc.vector.tensor_tensor(wlo[k][:], wst[:], whi[k][:],
                                                op=OP.subtract)
                        qst = stag.tile([128, L], F32, tag="stag",
                                        name=f"qst{k}")
                        nc.sync.dma_start(qst[:], qT[k * 128:(k + 1) * 128, :])
                        qh = qhl.tile([128, L], BF16, tag=f"qh{k}", name=f"qh{k}")
                        ql = qhl.tile([128, L], BF16, tag=f"ql{k}", name=f"ql{k}")
                        nc.vector.tensor_copy(qh[:], qst[:])
                        nc.vector.tensor_tensor(ql[:], qst[:], qh[:], op=OP.subtract)
                        qhs.append(qh)
                        qls.append(ql)
                    qh = qhs[k]
                    ql = qls[k]
                    for d in dks:
                        dsl = slice(d * 128, (d + 1) * 128)
                        for c in range(LCH):
                            csl = slice(c * LCW, (c + 1) * LCW)
                            ps = chunks[(d, c)][:]
                            nc.tensor.matmul(ps, whi[k][:, dsl], qh[:, csl],
                                             start=(k == 0), stop=False)
                            nc.tensor.matmul(ps, wlo[k][:, dsl], qh[:, csl],
                                             start=False, stop=False)
                            nc.tensor.matmul(ps, whi[k][:, dsl], ql[:, csl],
                                             start=False, stop=(k == KQ - 1))
                    if g0 == groups[0] and k == KQ - 1:
                        # kT loads queue behind group 0's inputs; split to
                        # bf16 hi/lo while group 1 computes
                        for kk in range(KD):
                            split_from_dram(khi[kk], klo[kk],
                                            kT, slice(kk * 128, (kk + 1) * 128),
                                            S, f"kt{kk}")
                for d in dks:
                    for c in range(LCH):
                        csl = slice(c * LCW, (c + 1) * LCW)
                        nc.scalar.activation(qwthi[d][:, csl], chunks[(d, c)][:],
                                             AF.Copy)
                        nc.vector.tensor_tensor(qwtlo[d][:, csl], chunks[(d, c)][:],
                                                qwthi[d][:, csl], op=OP.subtract)

        # ---- V load as f32r (phase-1 pools closed; space freed) ----
        vh = ctx.enter_context(tc.tile_pool(name="vh", bufs=1))
        vrt = []
        VST = min(DV, 512)
        for j in range(SJ):
            vr = vh.tile([128, DV], F32R, tag=f"vr{j}", name=f"vr{j}")
            for vc in range(DV // VST):
                vsl = slice(vc * VST, (vc + 1) * VST)
                st = vstag.tile([128, VST], F32, tag="vstag",
                                name=f"vstag{j}_{vc}")
                nc.sync.dma_start(st[:], v[j * 128:(j + 1) * 128, vsl])
                nc.scalar.activation(vr[:, vsl], st[:], AF.Copy)
            vrt.append(vr)

        # ---- phase 2/3, software-pipelined per l-tile ----
        # emission order: S(0), sm(0), [S(i), T/AV(i-1), sm(i)]..., T/AV(last)
        # so PE runs T/AV of tile i-1 while tile i's softmax latency resolves.
        with (
            tc.tile_pool(name="ep", bufs=2) as ep,
            tc.tile_pool(name="ptp", bufs=1) as ptp,
            tc.tile_pool(name="op", bufs=2) as op_pool,
            tc.tile_pool(name="scp", bufs=2) as scp,
            tc.tile_pool(name="ps_s", bufs=4, space="PSUM") as ps_s,
            tc.tile_pool(name="ps_t", bufs=2, space="PSUM") as ps_t,
            tc.tile_pool(name="ps_o", bufs=2, space="PSUM") as ps_o,
        ):
            st_chunks = {}
            st_e = {}
            st_rinv = {}

            def emit_S(i):
                lsl = slice(i * 128, (i + 1) * 128)
                schunks = [ps_s.tile([128, CH], F32, name=f"ssc{i}_{c}", tag="ssc")
                           for c in range(NCH)]
                for k in range(KD):
                    for c in range(NCH):
                        csl = slice(c * CH, (c + 1) * CH)
                        nc.tensor.matmul(schunks[c][:], qwthi[k][:, lsl],
                                         khi[k][:, csl],
                                         start=(k == 0), stop=False)
                        nc.tensor.matmul(schunks[c][:], qwtlo[k][:, lsl],
                                         khi[k][:, csl],
                                         start=False, stop=False)
                        nc.tensor.matmul(schunks[c][:], qwthi[k][:, lsl],
                                         klo[k][:, csl],
                                         start=False, stop=(k == KD - 1))
                st_chunks[i] = schunks

            def emit_softmax(i):
                schunks = st_chunks[i]
                scal = scp.tile([128, 8], F32, tag="scal", name=f"scal{i}")
                for c in range(NCH):
                    nc.vector.reduce_max(scal[:, c:c + 1], schunks[c][:],
                                         axis=AX.X, negate=True)
                nm = scal[:, NCH:NCH + 1]
                if NCH == 1:
                    nc.vector.tensor_copy(nm, scal[:, 0:1])
                else:
                    nc.vector.tensor_tensor(nm, scal[:, 0:1], scal[:, 1:2], op=OP.min)
                    for c in range(2, NCH):
                        nc.vector.tensor_tensor(nm, nm, scal[:, c:c + 1], op=OP.min)

                e = ep.tile([128, S], F32, tag="e", name=f"e{i}")
                sums = scp.tile([128, NCH + 2], F32, tag="sums", name=f"sums{i}")
                for c in range(NCH):
                    nc.scalar.activation(e[:, c * CH:(c + 1) * CH], schunks[c][:],
                                         AF.Exp, bias=nm,
                                         accum_out=sums[:, c:c + 1])
                if apply_mask:
                    # multiplicative mask after exp == additive -inf mask
                    # pre-softmax (invalid only for fully-masked rows)
                    nc.vector.tensor_tensor(e[:], e[:], m128[:], op=OP.mult)
                    nc.vector.reduce_sum(sums[:, NCH:NCH + 1], e[:], axis=AX.X)
                    tot = sums[:, NCH:NCH + 1]
                else:
                    tot = sums[:, NCH:NCH + 1]
                    if NCH == 1:
                        tot = sums[:, 0:1]
                    else:
                        nc.vector.tensor_tensor(tot, sums[:, 0:1], sums[:, 1:2],
                                                op=OP.add)
                        for c in range(2, NCH):
                            nc.vector.tensor_tensor(tot, tot, sums[:, c:c + 1],
                                                    op=OP.add)
                rinv = sums[:, NCH + 1:NCH + 2]
                nc.vector.reciprocal(rinv, tot)
                st_e[i] = e
                st_rinv[i] = rinv

            def emit_TAV(i, last=False):
                lsl = slice(i * 128, (i + 1) * 128)
                e = st_e[i]
                rinv = st_rinv[i]
                if last:
                    # final tile: normalize first (nothing left to overlap),
                    # so the tail is transpose->AV->copy only
                    nc.scalar.activation(e[:], e[:], AF.Copy, scale=rinv)
                    nc.sync.dma_start(p[lsl, :], e[:])
                # transpose UNNORMALIZED exp values; fold 1/rowsum into O copy
                ptr = []
                for j in range(SJ):
                    pst = ps_t.tile([128, 128], F32, name=f"pst{i}_{j}", tag="pst")
                    nc.tensor.transpose(pst[:], e[:, j * 128:(j + 1) * 128], ident[:])
                    pr = ptp.tile([128, 128], F32R, tag=f"ptr{j}", name=f"ptr{i}_{j}")
                    nc.vector.tensor_copy(pr[:], pst[:])
                    ptr.append(pr)

                pos = [ps_o.tile([128, OCW], F32, name=f"po{i}_{c}", tag="po")
                       for c in range(OCH)]
                for j in range(SJ):
                    for c in range(OCH):
                        nc.tensor.matmul(pos[c][:], ptr[j][:],
                                         vrt[j][:, c * OCW:(c + 1) * OCW],
                                         start=(j == 0), stop=(j == SJ - 1))
                for c in range(OCH):
                    osl = slice(c * OCW, (c + 1) * OCW)
                    ot = op_pool.tile([128, OCW], F32, tag="ot", name=f"ot{i}_{c}")
                    if last:
                        nc.vector.tensor_copy(ot[:], pos[c][:])
                    else:
                        nc.vector.tensor_scalar_mul(ot[:], pos[c][:], rinv)
                    nc.sync.dma_start(o[lsl, osl], ot[:])
                if not last:
                    # normalize P for the score output (off the PE critical path)
                    nc.scalar.activation(e[:], e[:], AF.Copy, scale=rinv)
                    nc.sync.dma_start(p[lsl, :], e[:])

            emit_S(0)
            emit_softmax(0)
            for i in range(1, LT):
                emit_S(i)
                emit_TAV(i - 1)
                emit_softmax(i)
            emit_TAV(LT - 1, last=True)

    nc.compile()
    return nc


_PROGRAM_CACHE = {}


def _get_program(L, S, DQ, DK, DV, apply_mask):
    key = (L, S, DQ, DK, DV, apply_mask)
    if key not in _PROGRAM_CACHE:
        _PROGRAM_CACHE[key] = _build_program(L, S, DQ, DK, DV, apply_mask)
    return _PROGRAM_CACHE[key]


def _run(query, key, value, W, mask, trace=False):
    n, l, dq = query.shape
    _, s, dk = key.shape
    dv = value.shape[2]
    assert n * 2 == N_CORES and l % (2 * 128) == 0
    L = l // 2

    apply_mask = not bool(np.all(mask))
    nc = _get_program(L, s, dq, dk, dv, apply_mask)

    in_maps = []
    for core in range(N_CORES):
        b, h = divmod(core, 2)
        im = {
            "qT": np.ascontiguousarray(
                query[b, h * L:(h + 1) * L, :].T).astype(np.float32, copy=False),
            "w": np.ascontiguousarray(W).astype(np.float32, copy=False),
            "kT": np.ascontiguousarray(key[b].T).astype(np.float32, copy=False),
            "v": np.ascontiguousarray(value[b]).astype(np.float32, copy=False),
        }
        if apply_mask:
            im["maskf"] = np.ascontiguousarray(
                mask[b].astype(np.float32)[None, :])
        in_maps.append(im)

    res = run_bass_kernel_spmd(nc, in_maps, core_ids=list(range(N_CORES)),
                               trace=trace)

    score = np.empty((n, l, s), dtype=np.float32)
    out = np.empty((n, l, dv), dtype=np.float32)
    for core in range(N_CORES):
        b, h = divmod(core, 2)
        score[b, h * L:(h + 1) * L, :] = res.results[core]["p"]
        out[b, h * L:(h + 1) * L, :] = res.results[core]["o"]
    return (out, score), res


def kernel(query, key, value, W, mask):
    (out, score), _ = _run(np.asarray(query), np.asarray(key), np.asarray(value),
                           np.asarray(W), np.asarray(mask))
    return (out, score)
